# revision 1
# baseline (speedup 1.0000x reference)
"""Causal self-attention with RoPE on 8 TRN2 NeuronCores.

Sharding: core c -> (batch b = c//4, head-group g = c%4; 4 heads of 128 each).
Tensor-parallel over heads x data-parallel over batch. After per-head
attention, the 4 cores of a batch AllGather their y^T shards, then each
core computes a disjoint 512-column slice of the output projection.

Layouts (all chosen so no on-chip transposes are ever needed):
  xT   [D, S]   = x[b].T                      (host-transposed)
  Q^T,K^T [128, S] per head  (from matmul: lhsT=W-block, rhs=xT)
  V    [S, 512] token-major  (from matmul: lhsT=xT-tile, rhs=Wv)
  S^T  [j, i] scores blocks -> softmax sums via ones-matmul on PE
  O^T  [c, i] accumulated in PSUM, normalized by 1/rowsum afterwards
  z^T  [512, S] output slice (host transposes back)

All matmuls run in float32r (~13-bit mantissa, 4x faster than fp32 on PE).
"""
from contextlib import ExitStack

import numpy as np

import concourse.bass as bass
import concourse.tile as tile
import concourse.mybir as mybir
from concourse import bacc, bass_utils

import os as _os
B = 2
S = int(_os.environ.get("K_S", "2048"))
D = int(_os.environ.get("K_D", "2048"))
NH, HD = 16, 128
HPC = 4                 # heads per core
EL = HPC * HD           # 512: local e-width per core
CH = 512                # i-chunk / s-chunk width
NCH = S // CH           # 4
DT = D // 128           # 16 d-tiles
ROPE_THETA = 10000.0
N_CORES = 8

F32 = mybir.dt.float32
F32R = mybir.dt.float32r
AF = mybir.ActivationFunctionType


def _build():
    nc = bacc.Bacc("TRN2", target_bir_lowering=False, debug=False,
                   enable_asserts=True, num_devices=N_CORES)
    xT = nc.dram_tensor("xT", [D, S], F32R, kind="ExternalInput").ap()
    wq = nc.dram_tensor("wq", [D, EL], F32R, kind="ExternalInput").ap()
    wk = nc.dram_tensor("wk", [D, EL], F32R, kind="ExternalInput").ap()
    wv = nc.dram_tensor("wv", [D, EL], F32R, kind="ExternalInput").ap()
    wp = nc.dram_tensor("wp", [D, EL], F32R, kind="ExternalInput").ap()
    cosq = nc.dram_tensor("cosq", [HD, S], F32, kind="ExternalInput").ap()
    sinq = nc.dram_tensor("sinq", [HD, S], F32, kind="ExternalInput").ap()
    cosk = nc.dram_tensor("cosk", [HD, S], F32, kind="ExternalInput").ap()
    sink = nc.dram_tensor("sink", [HD, S], F32, kind="ExternalInput").ap()
    tri = nc.dram_tensor("tri", [128, 128], F32, kind="ExternalInput").ap()
    ones = nc.dram_tensor("ones", [128, 1], F32R, kind="ExternalInput").ap()
    onesT = nc.dram_tensor("onesT", [1, 128], F32R, kind="ExternalInput").ap()
    zT = nc.dram_tensor("zT", [EL, S], F32, kind="ExternalOutput").ap()

    xTr = xT.rearrange("(t p) s -> t p s", p=128)
    wqr = wq.rearrange("(t p) e -> t p e", p=128)
    wkr = wk.rearrange("(t p) e -> t p e", p=128)
    wvr = wv.rearrange("(t p) e -> t p e", p=128)
    wpr = wp.rearrange("(t p) e -> t p e", p=128)

    with tile.TileContext(nc) as tc, \
         nc.allow_low_precision(reason="fp32r attention"), ExitStack() as ctx:
        if True:
            vres = ctx.enter_context(tc.tile_pool(name="vres", bufs=16))
            kres = ctx.enter_context(tc.tile_pool(name="kres", bufs=4))
            cpool = ctx.enter_context(tc.tile_pool(name="const", bufs=1))
            dram = ctx.enter_context(tc.tile_pool(name="dram", bufs=1, space="DRAM"))
            ps_mm = ctx.enter_context(tc.tile_pool(name="ps_mm", bufs=2, space="PSUM"))
            ps_sc = ctx.enter_context(tc.tile_pool(name="ps_sc", bufs=3, space="PSUM"))
            ps_o = ctx.enter_context(tc.tile_pool(name="ps_o", bufs=1, space="PSUM"))
            ps_r = ctx.enter_context(tc.tile_pool(name="ps_r", bufs=1, space="PSUM"))
            ps_b = ctx.enter_context(tc.tile_pool(name="ps_b", bufs=1, space="PSUM"))

            tri_t = cpool.tile([128, 128], F32)
            nc.sync.dma_start(tri_t[:], tri)
            ones_t = cpool.tile([128, 1], F32R)
            nc.sync.dma_start(ones_t[:], ones)
            onesT_t = cpool.tile([1, 128], F32R)
            nc.sync.dma_start(onesT_t[:], onesT)

            q_spill = dram.tile([EL, S], F32R)
            y_loc = [dram.tile([EL, CH], F32R, tag=f"yl{ci}", name=f"yl{ci}")
                     for ci in range(NCH)]
            y_full = [dram.tile([D, CH], F32R, tag=f"yf{ci}", name=f"yf{ci}")
                      for ci in range(NCH)]

            v_t = [vres.tile([128, EL], F32R, tag="v", name=f"v{st}")
                   for st in range(S // 128)]
            k_t = [kres.tile([HD, S], F32R, tag="k", name=f"k{h}")
                   for h in range(HPC)]

            # ---------------- pass 1: V = x @ Wv  (token-major) -------------
            with ExitStack() as vctx:
                p1w = vctx.enter_context(tc.tile_pool(name="p1", bufs=18))
                p1x = vctx.enter_context(tc.tile_pool(name="p1x", bufs=18))
                wv_t = []
                for dt in range(DT):
                    w = p1w.tile([128, EL], F32R, tag="w", name=f"wv{dt}")
                    nc.sync.dma_start(w[:], wvr[dt])
                    wv_t.append(w)
                for sc in range(NCH):
                    xc = []
                    for dt in range(DT):
                        xt = p1x.tile([128, CH], F32R, tag="x", name=f"x{sc}_{dt}")
                        nc.sync.dma_start(xt[:], xTr[dt][:, sc * CH:(sc + 1) * CH])
                        xc.append(xt)
                    for st in range(CH // 128):
                        ps = ps_mm.tile([128, EL], F32)
                        for dt in range(DT):
                            nc.tensor.matmul(
                                ps[:], xc[dt][:, st * 128:(st + 1) * 128], wv_t[dt][:],
                                start=(dt == 0), stop=(dt == DT - 1))
                        nc.scalar.copy(v_t[sc * 4 + st][:], ps[:])

            # ------------- passes 2/3: K^T then Q^T (+RoPE), Q spilled -------
            def kq_pass(wsrc, cos_src, sin_src, is_q, tagp):
                with ExitStack() as kctx:
                    pw = kctx.enter_context(tc.tile_pool(name=f"{tagp}w", bufs=18))
                    px = kctx.enter_context(tc.tile_pool(name=f"{tagp}x", bufs=18))
                    pcs = kctx.enter_context(tc.tile_pool(name=f"{tagp}cs", bufs=2))
                    pt = kctx.enter_context(tc.tile_pool(name=f"{tagp}t", bufs=3))
                    w_t = []
                    for dt in range(DT):
                        w = pw.tile([128, EL], F32R, tag="w", name=f"{tagp}w{dt}")
                        nc.sync.dma_start(w[:], wsrc[dt])
                        w_t.append(w)
                    for sc in range(NCH):
                        xc = []
                        for dt in range(DT):
                            xt = px.tile([128, CH], F32R, tag="x",
                                         name=f"{tagp}x{sc}_{dt}")
                            nc.sync.dma_start(
                                xt[:], xTr[dt][:, sc * CH:(sc + 1) * CH])
                            xc.append(xt)
                        cs = pcs.tile([128, CH], F32, tag="cs")
                        nc.sync.dma_start(cs[:], cos_src[:, sc * CH:(sc + 1) * CH])
                        sn = pcs.tile([128, CH], F32, tag="sn")
                        nc.sync.dma_start(sn[:], sin_src[:, sc * CH:(sc + 1) * CH])
                        for h in range(HPC):
                            ps = ps_mm.tile([HD, CH], F32)
                            for dt in range(DT):
                                nc.tensor.matmul(
                                    ps[:], w_t[dt][:, h * HD:(h + 1) * HD], xc[dt][:],
                                    start=(dt == 0), stop=(dt == DT - 1))
                            pre = pt.tile([128, CH], F32, tag="pre")
                            nc.scalar.copy(pre[:], ps[:])
                            # out = pre*cos + rot(pre)*sin (sign folded into sin)
                            rot = pt.tile([128, CH], F32, tag="rot")
                            nc.sync.dma_start(rot[0:64, :], pre[64:128, :])
                            nc.sync.dma_start(rot[64:128, :], pre[0:64, :])
                            t1 = pt.tile([128, CH], F32, tag="t1")
                            nc.vector.tensor_mul(t1[:], pre[:], cs[:])
                            t2 = pt.tile([128, CH], F32, tag="rot2")
                            nc.vector.tensor_mul(t2[:], rot[:], sn[:])
                            if is_q:
                                qro = pt.tile([128, CH], F32R, tag="qro")
                                nc.vector.tensor_add(qro[:], t1[:], t2[:])
                                nc.sync.dma_start(
                                    q_spill[h * HD:(h + 1) * HD,
                                            sc * CH:(sc + 1) * CH], qro[:])
                            else:
                                nc.vector.tensor_add(
                                    k_t[h][:, sc * CH:(sc + 1) * CH], t1[:], t2[:])

            kq_pass(wkr, cosk, sink, False, "pk")
            kq_pass(wqr, cosq, sinq, True, "pq")

            # ------- pass 3: causal attention + AG + projection (pipelined) --
            with ExitStack() as actx:
                p3q = actx.enter_context(tc.tile_pool(name="p3q", bufs=5))
                p3p = actx.enter_context(tc.tile_pool(name="p3p", bufs=3))
                p3o = actx.enter_context(tc.tile_pool(name="p3o", bufs=4))
                p3y = actx.enter_context(tc.tile_pool(name="p3y", bufs=2))
                p3r = actx.enter_context(tc.tile_pool(name="p3r", bufs=2))
                p4w = actx.enter_context(tc.tile_pool(name="p4w", bufs=16))
                p4y = actx.enter_context(tc.tile_pool(name="p4y", bufs=18))
                p4z = actx.enter_context(tc.tile_pool(name="p4z", bufs=3))
                wp_t = []
                for et in range(DT):
                    w = p4w.tile([128, EL], F32R, tag="w", name=f"wp{et}")
                    nc.sync.dma_start(w[:], wpr[et])
                    wp_t.append(w)

                def proj_chunk(sc):
                    yfr = y_full[sc][:].rearrange("(t p) s -> t p s", p=128)
                    yc = []
                    for et in range(DT):
                        yt = p4y.tile([128, CH], F32R, tag="y", name=f"yg{sc}_{et}")
                        nc.sync.dma_start(yt[:], yfr[et])
                        yc.append(yt)
                    for ep in range(EL // 128):
                        ps = ps_mm.tile([128, CH], F32)
                        for et in range(DT):
                            nc.tensor.matmul(
                                ps[:], wp_t[et][:, ep * 128:(ep + 1) * 128], yc[et][:],
                                start=(et == 0), stop=(et == DT - 1))
                        zt = p4z.tile([128, CH], F32, tag="z")
                        nc.scalar.copy(zt[:], ps[:])
                        nc.sync.dma_start(
                            zT[ep * 128:(ep + 1) * 128, sc * CH:(sc + 1) * CH], zt[:])

                for ci in range(NCH):
                    qc = []
                    for h in range(HPC):
                        qt = p3q.tile([HD, CH], F32R, tag="q", name=f"q{ci}_{h}")
                        nc.sync.dma_start(
                            qt[:], q_spill[h * HD:(h + 1) * HD, ci * CH:(ci + 1) * CH])
                        qc.append(qt)
                    rall = p3r.tile([HPC, CH], F32, tag="rall")
                    o_sb = []
                    n_jt = 4 * ci + 4
                    for h in range(HPC):
                        o_ps = ps_o.tile([HD, CH], F32)
                        r_ps = ps_r.tile([1, CH], F32)
                        for jt in range(n_jt):
                            diag = jt - 4 * ci
                            off = 128 * diag if diag > 0 else 0
                            s_ps = ps_sc.tile([128, CH], F32)
                            nc.tensor.matmul(
                                s_ps[:, off:], k_t[h][:, jt * 128:(jt + 1) * 128],
                                qc[h][:, off:], start=True, stop=True)
                            p = p3p.tile([128, CH], F32R, tag="p")
                            nc.scalar.activation(p[:, off:], s_ps[:, off:], AF.Exp)
                            if diag >= 0:
                                nc.vector.tensor_mul(
                                    p[:, off:off + 128], p[:, off:off + 128], tri_t[:])
                            nc.tensor.matmul(
                                o_ps[:, off:], v_t[jt][:, h * HD:(h + 1) * HD],
                                p[:, off:], start=(jt == 0), stop=(jt == n_jt - 1))
                            nc.tensor.matmul(
                                r_ps[:, off:], ones_t[:], p[:, off:],
                                start=(jt == 0), stop=(jt == n_jt - 1))
                        rsb = p3r.tile([1, CH], F32, tag="rsb")
                        nc.vector.tensor_copy(rsb[:], r_ps[:])
                        nc.sync.dma_start(rall[h:h + 1, :], rsb[:])
                        ot = p3o.tile([HD, CH], F32R, tag="o", name=f"o{ci}_{h}")
                        nc.vector.tensor_copy(ot[:], o_ps[:])
                        o_sb.append(ot)
                    rinv = p3r.tile([HPC, CH], F32R, tag="rinv")
                    nc.vector.reciprocal(rinv[:], rall[:])
                    for h in range(HPC):
                        rrow = p3r.tile([1, CH], F32R, tag="rrow")
                        nc.sync.dma_start(rrow[:], rinv[h:h + 1, :])
                        b_ps = ps_b.tile([128, CH], F32)
                        nc.tensor.matmul(b_ps[:], onesT_t[:], rrow[:],
                                         start=True, stop=True)
                        yt = p3y.tile([HD, CH], F32R, tag="y")
                        nc.vector.tensor_mul(yt[:], o_sb[h][:], b_ps[:])
                        nc.sync.dma_start(
                            y_loc[ci][h * HD:(h + 1) * HD, :], yt[:])
                    # AllGather this chunk within the batch group (pipelines
                    # with the next chunk's attention and with the projection)
                    nc.gpsimd.collective_compute(
                        "AllGather", mybir.AluOpType.bypass,
                        replica_groups=[[0, 1, 2, 3], [4, 5, 6, 7]],
                        ins=[y_loc[ci].opt()], outs=[y_full[ci].opt()])
                # projection emitted after attention (lower scheduler priority
                # so it fills idle engine time), but pools coexist so nothing
                # forces it to wait for the attention phase to finish
                for sc in range(NCH):
                    proj_chunk(sc)
    nc.compile()
    return nc


def _tables():
    inv_freq = 1.0 / (ROPE_THETA ** (np.arange(0, HD, 2, dtype=np.float64) / HD))
    pos = np.arange(S, dtype=np.float64)
    f_half = np.outer(inv_freq, pos)                  # [64, S]
    freqs = np.concatenate([f_half, f_half], axis=0)  # [HD, S]
    # match reference numerics: cos/sin computed in float32 domain
    emb32 = freqs.astype(np.float32)
    cos_t = np.cos(emb32)
    sin_t = np.sin(emb32)
    scale = np.float32(HD ** -0.5)
    sgn = np.where(np.arange(HD) < HD // 2, -1.0, 1.0).astype(np.float32)[:, None]
    cosq = (cos_t * scale).astype(np.float32)
    sinq = (sin_t * sgn * scale).astype(np.float32)
    cosk = cos_t.astype(np.float32)
    sink = (sin_t * sgn).astype(np.float32)
    return cosq, sinq, cosk, sink


_NC_CACHE = {}


def _get_nc():
    if "nc" not in _NC_CACHE:
        _NC_CACHE["nc"] = _build()
    return _NC_CACHE["nc"]


def make_in_maps(x, W_attn, W_proj):
    x = np.asarray(x, dtype=np.float32)
    W_attn = np.asarray(W_attn, dtype=np.float32)
    W_proj = np.asarray(W_proj, dtype=np.float32)
    cosq, sinq, cosk, sink = _tables()
    tri = np.triu(np.ones((128, 128), np.float32))   # [jj, ii]: keep jj <= ii
    ones = np.ones((128, 1), np.float32)
    onesT = np.ones((1, 128), np.float32)
    in_maps = []
    for c in range(N_CORES):
        b, g = divmod(c, HPC)
        in_maps.append({
            "xT": np.ascontiguousarray(x[b].T),
            "wq": np.ascontiguousarray(W_attn[:, g * EL:(g + 1) * EL]),
            "wk": np.ascontiguousarray(W_attn[:, D + g * EL:D + (g + 1) * EL]),
            "wv": np.ascontiguousarray(W_attn[:, 2 * D + g * EL:2 * D + (g + 1) * EL]),
            "wp": np.ascontiguousarray(W_proj[:, g * EL:(g + 1) * EL]),
            "cosq": cosq, "sinq": sinq, "cosk": cosk, "sink": sink,
            "tri": tri, "ones": ones, "onesT": onesT,
        })
    return in_maps


def assemble(results):
    out = np.empty((B, S, D), dtype=np.float32)
    for c in range(N_CORES):
        b, g = divmod(c, HPC)
        out[b, :, g * EL:(g + 1) * EL] = results[c]["zT"].T
    return out


def kernel(x, W_attn, W_proj):
    nc = _get_nc()
    in_maps = make_in_maps(x, W_attn, W_proj)
    res = bass_utils.run_bass_kernel_spmd(
        nc, in_maps, core_ids=list(range(N_CORES)), trace=False)
    return assemble(res.results)


if __name__ == "__main__":
    rng = np.random.default_rng(0)
    x = rng.standard_normal((B, S, D)).astype(np.float32)
    W_attn = (rng.standard_normal((D, 3 * D)) * D ** -0.5).astype(np.float32)
    W_proj = (rng.standard_normal((D, D)) * D ** -0.5).astype(np.float32)
    out = kernel(x, W_attn, W_proj)
    print("out", out.shape, out.dtype, np.abs(out).mean())



# revision 9
# speedup vs baseline: 1.7402x; 1.7402x over previous
"""Causal self-attention with RoPE on 8 TRN2 NeuronCores.

Sharding: core c -> (batch b = c//4, head-group g = c%4; 4 heads of 128 each).
Tensor-parallel over heads x data-parallel over batch.

Single fused pipeline per core, one pass over x. For each 512-token chunk ci:
  QKV matmuls (+RoPE on Q,K) -> causal attention over kv-chunks <= ci
  -> project own heads through own W_proj row-block into a full-D partial z
  -> per-chunk ReduceScatter(add) within the batch group writes this core's
     final 512-dim slice of z^T. Only the last chunk's collective sits on the
     critical path (~28us) vs. the serial AllGather tail it replaces.

All matmul operands are fp16 (1 PE cycle/row, same as bf16, 10-bit mantissa).
Softmax stats, RoPE math and PSUM accumulation stay fp32. PSUM->SBUF copies
run on the otherwise-idle Pool engine; the 1/rowsum broadcast uses Pool's
partition_broadcast instead of a PE matmul.

Layouts (no on-chip transposes needed):
  xT   [D, S]   = x[b].T                      (host-transposed)
  Q^T,K^T [128, S] per head  (from matmul: lhsT=W-block, rhs=xT)
  V    [S, 512] token-major  (from matmul: lhsT=xT-tile, rhs=Wv)
  S^T  [j, i] scores blocks -> softmax sums via ones-matmul on PE
  O^T  [c, i] accumulated in PSUM, normalized by 1/rowsum afterwards
  zp   [D, S-chunk] partial projection, ReduceScatter -> zc[ci] [512, 512]
"""
from contextlib import ExitStack

import numpy as np

import concourse.bass as bass
import concourse.tile as tile
import concourse.mybir as mybir
from concourse import bacc, bass_utils

B = 2
S = 2048
D = 2048
NH, HD = 16, 128
HPC = 4                 # heads per core
EL = HPC * HD           # 512: local e-width per core
CH = 512                # token-chunk width
NCH = S // CH           # 4
DT = D // 128           # 16 d-tiles
KT = EL // 128          # 4 k-tiles of the local proj contraction
ROPE_THETA = 10000.0
N_CORES = 8
GROUPS = [[0, 1, 2, 3], [4, 5, 6, 7]]

F32 = mybir.dt.float32
F16 = mybir.dt.float16
AF = mybir.ActivationFunctionType


def _build():
    nc = bacc.Bacc("TRN2", target_bir_lowering=False, debug=False,
                   enable_asserts=True, num_devices=N_CORES)
    xT = nc.dram_tensor("xT", [D, S], F16, kind="ExternalInput").ap()
    wq = nc.dram_tensor("wq", [D, EL], F16, kind="ExternalInput").ap()
    wk = nc.dram_tensor("wk", [D, EL], F16, kind="ExternalInput").ap()
    wv = nc.dram_tensor("wv", [D, EL], F16, kind="ExternalInput").ap()
    wp = nc.dram_tensor("wp", [EL, D], F16, kind="ExternalInput").ap()
    cosq = nc.dram_tensor("cosq", [HD, S], F32, kind="ExternalInput").ap()
    sinq = nc.dram_tensor("sinq", [HD, S], F32, kind="ExternalInput").ap()
    cosk = nc.dram_tensor("cosk", [HD, S], F32, kind="ExternalInput").ap()
    sink = nc.dram_tensor("sink", [HD, S], F32, kind="ExternalInput").ap()
    tri = nc.dram_tensor("tri", [128, 128], F16, kind="ExternalInput").ap()
    ones = nc.dram_tensor("ones", [128, 1], F16, kind="ExternalInput").ap()
    zc = nc.dram_tensor("zc", [NCH, EL, CH], F16, kind="ExternalOutput").ap()

    xTr = xT.rearrange("(t p) s -> t p s", p=128)
    wqr = wq.rearrange("(t p) e -> t p e", p=128)
    wkr = wk.rearrange("(t p) e -> t p e", p=128)
    wvr = wv.rearrange("(t p) e -> t p e", p=128)
    wpr = wp.rearrange("(k p) d -> k p d", p=128)

    with tile.TileContext(nc) as tc, \
         nc.allow_low_precision(reason="fp16 attention"), ExitStack() as ctx:
        cpool = ctx.enter_context(tc.tile_pool(name="const", bufs=1))
        wpool = ctx.enter_context(tc.tile_pool(name="wts", bufs=1))
        xpool = ctx.enter_context(tc.tile_pool(name="xc", bufs=20))
        kres = ctx.enter_context(tc.tile_pool(name="kres", bufs=4))
        vres = ctx.enter_context(tc.tile_pool(name="vres", bufs=16))
        qpool = ctx.enter_context(tc.tile_pool(name="qp", bufs=8))
        rpool = ctx.enter_context(tc.tile_pool(name="rope", bufs=2))
        ppool = ctx.enter_context(tc.tile_pool(name="pp", bufs=4))
        opool = ctx.enter_context(tc.tile_pool(name="op", bufs=4))
        spool = ctx.enter_context(tc.tile_pool(name="sm", bufs=3))
        ypool = ctx.enter_context(tc.tile_pool(name="yp", bufs=8))
        zpool = ctx.enter_context(tc.tile_pool(name="zp", bufs=4))
        dram = ctx.enter_context(tc.tile_pool(name="dram", bufs=1, space="DRAM"))
        ps_mm = ctx.enter_context(tc.tile_pool(name="ps_mm", bufs=2, space="PSUM"))
        ps_sc = ctx.enter_context(tc.tile_pool(name="ps_sc", bufs=2, space="PSUM"))
        ps_o = ctx.enter_context(tc.tile_pool(name="ps_o", bufs=2, space="PSUM"))
        ps_r = ctx.enter_context(tc.tile_pool(name="ps_r", bufs=2, space="PSUM"))

        # ---- constants + weights (DMA order = priority on the sync queue) --
        tri_t = cpool.tile([128, 128], F16)
        nc.sync.dma_start(tri_t[:], tri)
        ones_t = cpool.tile([128, 1], F16)
        nc.sync.dma_start(ones_t[:], ones)

        wk_t = []
        for dt in range(DT):
            w = wpool.tile([128, EL], F16, name=f"wk{dt}")
            nc.sync.dma_start(w[:], wkr[dt])
            wk_t.append(w)
        csk = cpool.tile([HD, S], F32, name="cosk")
        nc.sync.dma_start(csk[:], cosk)
        snk = cpool.tile([HD, S], F32, name="sink")
        nc.sync.dma_start(snk[:], sink)
        wq_t = []
        for dt in range(DT):
            w = wpool.tile([128, EL], F16, name=f"wq{dt}")
            nc.sync.dma_start(w[:], wqr[dt])
            wq_t.append(w)
        csq = cpool.tile([HD, S], F32, name="cosq")
        nc.sync.dma_start(csq[:], cosq)
        snq = cpool.tile([HD, S], F32, name="sinq")
        nc.sync.dma_start(snq[:], sinq)
        wv_t = []
        for dt in range(DT):
            w = wpool.tile([128, EL], F16, name=f"wv{dt}")
            nc.sync.dma_start(w[:], wvr[dt])
            wv_t.append(w)
        wp_t = []
        for kk in range(KT):
            w = wpool.tile([128, D], F16, name=f"wp{kk}")
            nc.sync.dma_start(w[:], wpr[kk])
            wp_t.append(w)

        k_t = [kres.tile([HD, S], F16, tag="k", name=f"k{h}")
               for h in range(HPC)]
        v_t = [vres.tile([128, EL], F16, tag="v", name=f"v{st}")
               for st in range(S // 128)]
        z_part = [dram.tile([D, CH], F16, tag=f"zp{ci}", name=f"zp{ci}")
                  for ci in range(NCH)]
        z_red = [dram.tile([EL, CH], F16, tag=f"zr{ci}", name=f"zr{ci}")
                 for ci in range(NCH)]

        def rope_qk(ps, cs, sn, out_tile, sl):
            """out = ps*cos + rotate_half(ps)*sin (sign folded into sin)."""
            pre = rpool.tile([128, CH], F32, tag="pre")
            nc.vector.tensor_copy(pre[:], ps[:])
            rot = rpool.tile([128, CH], F32, tag="rot")
            nc.sync.dma_start(rot[0:64, :], pre[64:128, :])
            nc.sync.dma_start(rot[64:128, :], pre[0:64, :])
            t1 = rpool.tile([128, CH], F32, tag="t1")
            nc.vector.tensor_mul(t1[:], pre[:], cs)
            t2 = rpool.tile([128, CH], F32, tag="t2")
            nc.vector.tensor_mul(t2[:], rot[:], sn)
            nc.vector.tensor_add(out_tile[:, sl], t1[:], t2[:])

        for ci in range(NCH):
            tsl = slice(ci * CH, (ci + 1) * CH)
            xc = []
            for dt in range(DT):
                xt = xpool.tile([128, CH], F16, tag="x", name=f"x{ci}_{dt}")
                nc.sync.dma_start(xt[:], xTr[dt][:, tsl])
                xc.append(xt)

            # ---------------- K chunk (+RoPE) ----------------
            for h in range(HPC):
                ps = ps_mm.tile([HD, CH], F32)
                for dt in range(DT):
                    nc.tensor.matmul(
                        ps[:], wk_t[dt][:, h * HD:(h + 1) * HD], xc[dt][:],
                        start=(dt == 0), stop=(dt == DT - 1))
                rope_qk(ps, csk[:, tsl], snk[:, tsl], k_t[h], tsl)

            # ---------------- Q chunk (+RoPE) ----------------
            qc = []
            for h in range(HPC):
                ps = ps_mm.tile([HD, CH], F32)
                for dt in range(DT):
                    nc.tensor.matmul(
                        ps[:], wq_t[dt][:, h * HD:(h + 1) * HD], xc[dt][:],
                        start=(dt == 0), stop=(dt == DT - 1))
                qt = qpool.tile([HD, CH], F16, tag="q", name=f"q{ci}_{h}")
                rope_qk(ps, csq[:, tsl], snq[:, tsl], qt, slice(None))
                qc.append(qt)

            # ---------------- V chunk (token-major) ----------------
            for st in range(CH // 128):
                ps = ps_mm.tile([128, EL], F32)
                for dt in range(DT):
                    nc.tensor.matmul(
                        ps[:], xc[dt][:, st * 128:(st + 1) * 128], wv_t[dt][:],
                        start=(dt == 0), stop=(dt == DT - 1))
                nc.scalar.copy(v_t[ci * 4 + st][:], ps[:])

            # ---------------- causal attention for this chunk ----------------
            y_sb = []
            n_jt = 4 * ci + 4
            for h in range(HPC):
                o_ps = ps_o.tile([HD, CH], F32)
                r_ps = ps_r.tile([1, CH], F32)
                for jt in range(n_jt):
                    diag = jt - 4 * ci
                    off = 128 * diag if diag > 0 else 0
                    s_ps = ps_sc.tile([128, CH], F32)
                    nc.tensor.matmul(
                        s_ps[:, off:], k_t[h][:, jt * 128:(jt + 1) * 128],
                        qc[h][:, off:], start=True, stop=True)
                    p = ppool.tile([128, CH], F16, tag="p")
                    nc.scalar.activation(p[:, off:], s_ps[:, off:], AF.Exp)
                    if diag >= 0:
                        nc.vector.tensor_mul(
                            p[:, off:off + 128], p[:, off:off + 128], tri_t[:])
                    nc.tensor.matmul(
                        o_ps[:, off:], v_t[jt][:, h * HD:(h + 1) * HD],
                        p[:, off:], start=(jt == 0), stop=(jt == n_jt - 1))
                    nc.tensor.matmul(
                        r_ps[:, off:], ones_t[:], p[:, off:],
                        start=(jt == 0), stop=(jt == n_jt - 1))
                rinv = spool.tile([1, CH], F16, tag="rinv")
                nc.vector.reciprocal(rinv[:], r_ps[:])
                bcast = spool.tile([128, CH], F16, tag="bcast")
                nc.gpsimd.partition_broadcast(bcast[:], rinv[:], channels=128)
                ot = opool.tile([HD, CH], F16, tag="o", name=f"o{ci}_{h}")
                nc.vector.tensor_copy(ot[:], o_ps[:])
                yt = ypool.tile([HD, CH], F16, tag="y", name=f"y{ci}_{h}")
                nc.vector.tensor_mul(yt[:], ot[:], bcast[:])
                y_sb.append(yt)

            # ------- partial projection: zp[ci] = Wp[g-rows]^T @ y -------
            for d2 in range(DT):
                ps = ps_mm.tile([128, CH], F32)
                for kk in range(KT):
                    nc.tensor.matmul(
                        ps[:], wp_t[kk][:, d2 * 128:(d2 + 1) * 128], y_sb[kk][:],
                        start=(kk == 0), stop=(kk == KT - 1))
                zt = zpool.tile([128, CH], F16, tag="z")
                nc.scalar.copy(zt[:], ps[:])
                nc.sync.dma_start(
                    z_part[ci][d2 * 128:(d2 + 1) * 128, :], zt[:])

            # ------- ReduceScatter: sum partials, keep own 512-dim slice ----
            nc.gpsimd.collective_compute(
                "ReduceScatter", mybir.AluOpType.add,
                replica_groups=GROUPS,
                ins=[z_part[ci].opt()], outs=[z_red[ci].opt()])
            nc.sync.dma_start(zc[ci], z_red[ci][:])
    nc.compile()
    return nc


def _tables():
    inv_freq = 1.0 / (ROPE_THETA ** (np.arange(0, HD, 2, dtype=np.float64) / HD))
    pos = np.arange(S, dtype=np.float64)
    f_half = np.outer(inv_freq, pos)                  # [64, S]
    freqs = np.concatenate([f_half, f_half], axis=0)  # [HD, S]
    # match reference numerics: cos/sin computed in float32 domain
    emb32 = freqs.astype(np.float32)
    cos_t = np.cos(emb32)
    sin_t = np.sin(emb32)
    scale = np.float32(HD ** -0.5)
    sgn = np.where(np.arange(HD) < HD // 2, -1.0, 1.0).astype(np.float32)[:, None]
    cosq = (cos_t * scale).astype(np.float32)
    sinq = (sin_t * sgn * scale).astype(np.float32)
    cosk = cos_t.astype(np.float32)
    sink = (sin_t * sgn).astype(np.float32)
    return cosq, sinq, cosk, sink


_NC_CACHE = {}


def _get_nc():
    if "nc" not in _NC_CACHE:
        _NC_CACHE["nc"] = _build()
    return _NC_CACHE["nc"]


def make_in_maps(x, W_attn, W_proj):
    x = np.asarray(x, dtype=np.float32)
    W_attn = np.asarray(W_attn, dtype=np.float32)
    W_proj = np.asarray(W_proj, dtype=np.float32)
    cosq, sinq, cosk, sink = _tables()
    tri = np.triu(np.ones((128, 128), np.float16))   # [jj, ii]: keep jj <= ii
    ones = np.ones((128, 1), np.float16)
    in_maps = []
    for c in range(N_CORES):
        b, g = divmod(c, HPC)
        in_maps.append({
            "xT": np.ascontiguousarray(x[b].T).astype(np.float16),
            "wq": W_attn[:, g * EL:(g + 1) * EL].astype(np.float16),
            "wk": W_attn[:, D + g * EL:D + (g + 1) * EL].astype(np.float16),
            "wv": W_attn[:, 2 * D + g * EL:2 * D + (g + 1) * EL].astype(np.float16),
            "wp": W_proj[g * EL:(g + 1) * EL, :].astype(np.float16),
            "cosq": cosq, "sinq": sinq, "cosk": cosk, "sink": sink,
            "tri": tri, "ones": ones,
        })
    return in_maps


def assemble(results):
    out = np.empty((B, S, D), dtype=np.float32)
    for c in range(N_CORES):
        b, g = divmod(c, HPC)
        zcs = np.asarray(results[c]["zc"], dtype=np.float32)
        for ci in range(NCH):
            out[b, ci * CH:(ci + 1) * CH, g * EL:(g + 1) * EL] = zcs[ci].T
    return out


def kernel(x, W_attn, W_proj):
    nc = _get_nc()
    in_maps = make_in_maps(x, W_attn, W_proj)
    res = bass_utils.run_bass_kernel_spmd(
        nc, in_maps, core_ids=list(range(N_CORES)), trace=False)
    return assemble(res.results)


if __name__ == "__main__":
    rng = np.random.default_rng(0)
    x = rng.standard_normal((B, S, D)).astype(np.float32)
    W_attn = (rng.standard_normal((D, 3 * D)) * D ** -0.5).astype(np.float32)
    W_proj = (rng.standard_normal((D, D)) * D ** -0.5).astype(np.float32)
    out = kernel(x, W_attn, W_proj)
    print("out", out.shape, out.dtype, np.abs(out).mean())


# revision 19
# speedup vs baseline: 2.0914x; 1.2018x over previous
"""Causal self-attention with RoPE on 8 TRN2 NeuronCores.

Sharding: core c -> (batch b = c//4, head-group g = c%4; 4 heads of 128 each).
Tensor-parallel over heads x data-parallel over batch.

Single fused pipeline per core, one pass over x. For each 512-token chunk ci:
  QKV matmuls (+RoPE on Q,K) -> causal attention over kv-chunks <= ci
  -> project own heads through own W_proj row-block into a full-D partial z
  -> per-chunk ReduceScatter(add) within the batch group produces this core's
     final 512-dim slice of z^T. Only the last chunk's collective sits on the
     critical path (~28us) vs. the serial AllGather tail it replaces.

All matmul operands are fp16 (1 PE cycle/row, same as bf16, 10-bit mantissa).
PSUM accumulation and softmax statistics stay fp32.

DMA queues are specialized to avoid head-of-line blocking of the input
prefetch stream: sync = input loads only (big batched transfers, halved so
the first matmuls of a chain can start early), DVE = RoPE rotate-half
SBUF-to-SBUF swaps, Act = z-partial spills (their producer is the Act copy),
Pool = collectives. The 1/rowsum broadcast runs on Pool's partition_broadcast
instead of a PE matmul.

Layouts (no on-chip transposes needed):
  xT   [D, S]   = x[b].T                      (host-transposed)
  Q^T,K^T [128, S] per head  (from matmul: lhsT=W-block, rhs=xT)
  V    [S, 512] token-major  (from matmul: lhsT=xT-tile, rhs=Wv)
  S^T  [j, i] scores blocks -> softmax sums via ones-matmul on PE
  O^T  [c, i] accumulated in PSUM, normalized by 1/rowsum afterwards
  zp   [D, S-chunk] partial projection, ReduceScatter -> z_red [512, 512]
"""
from contextlib import ExitStack

import numpy as np

import concourse.bass as bass
import concourse.tile as tile
import concourse.mybir as mybir
from concourse import bacc, bass_utils

B = 2
S = 2048
D = 2048
NH, HD = 16, 128
HPC = 4                 # heads per core
EL = HPC * HD           # 512: local e-width per core
CH = 512                # token-chunk width
NCH = S // CH           # 4
DT = D // 128           # 16 d-tiles
KT = EL // 128          # 4 k-tiles of the local proj contraction
ROPE_THETA = 10000.0
N_CORES = 8
GROUPS = [[0, 1, 2, 3], [4, 5, 6, 7]]

F32 = mybir.dt.float32
F16 = mybir.dt.float16
AF = mybir.ActivationFunctionType


def _build():
    nc = bacc.Bacc("TRN2", target_bir_lowering=False, debug=False,
                   enable_asserts=True, num_devices=N_CORES)
    xT = nc.dram_tensor("xT", [D, S], F16, kind="ExternalInput").ap()
    wq = nc.dram_tensor("wq", [D, EL], F16, kind="ExternalInput").ap()
    wk = nc.dram_tensor("wk", [D, EL], F16, kind="ExternalInput").ap()
    wv = nc.dram_tensor("wv", [D, EL], F16, kind="ExternalInput").ap()
    wp = nc.dram_tensor("wp", [EL, D], F16, kind="ExternalInput").ap()
    cosq = nc.dram_tensor("cosq", [HD, S], F16, kind="ExternalInput").ap()
    sinq = nc.dram_tensor("sinq", [HD, S], F16, kind="ExternalInput").ap()
    cosk = nc.dram_tensor("cosk", [HD, S], F16, kind="ExternalInput").ap()
    sink = nc.dram_tensor("sink", [HD, S], F16, kind="ExternalInput").ap()
    tri = nc.dram_tensor("tri", [128, 128], F16, kind="ExternalInput").ap()
    ones = nc.dram_tensor("ones", [128, 1], F16, kind="ExternalInput").ap()
    zc = nc.dram_tensor("zc", [NCH, EL, CH], F16, kind="ExternalOutput").ap()

    # [p, t, e] views: 128 partitions, d-tiles stacked along a middle dim
    wqv = wq.rearrange("(t p) e -> p t e", p=128)
    wkv = wk.rearrange("(t p) e -> p t e", p=128)
    wvv = wv.rearrange("(t p) e -> p t e", p=128)
    wpv = wp.rearrange("(k p) d -> p k d", p=128)

    HW = 8 * CH          # half-width of a batched x / qkv-weight transfer

    with tile.TileContext(nc) as tc, \
         nc.allow_low_precision(reason="fp16 attention"), ExitStack() as ctx:
        cpool = ctx.enter_context(tc.tile_pool(name="const", bufs=1))
        wpool = ctx.enter_context(tc.tile_pool(name="wts", bufs=1))
        xpool = ctx.enter_context(tc.tile_pool(name="xc", bufs=2))
        kres = ctx.enter_context(tc.tile_pool(name="kres", bufs=4))
        vres = ctx.enter_context(tc.tile_pool(name="vres", bufs=16))
        qpool = ctx.enter_context(tc.tile_pool(name="qp", bufs=6))
        rpool = ctx.enter_context(tc.tile_pool(name="rope", bufs=2))
        ppool = ctx.enter_context(tc.tile_pool(name="pp", bufs=4))
        opool = ctx.enter_context(tc.tile_pool(name="op", bufs=4))
        spool = ctx.enter_context(tc.tile_pool(name="sm", bufs=3))
        ypool = ctx.enter_context(tc.tile_pool(name="yp", bufs=5))
        zpool = ctx.enter_context(tc.tile_pool(name="zp", bufs=2))
        dram = ctx.enter_context(tc.tile_pool(name="dram", bufs=1, space="DRAM"))
        ps_mm = ctx.enter_context(tc.tile_pool(name="ps_mm", bufs=2, space="PSUM"))
        ps_sc = ctx.enter_context(tc.tile_pool(name="ps_sc", bufs=3, space="PSUM"))
        ps_o = ctx.enter_context(tc.tile_pool(name="ps_o", bufs=2, space="PSUM"))
        ps_r = ctx.enter_context(tc.tile_pool(name="ps_r", bufs=1, space="PSUM"))

        # ---- prefetch stream (sync queue order == arrival priority) ----
        tri_t = cpool.tile([128, 128], F16)
        nc.sync.dma_start(tri_t[:], tri)
        ones_t = cpool.tile([128, 1], F16)
        nc.sync.dma_start(ones_t[:], ones)

        wk_lo = wpool.tile([128, HW], F16, name="wk_lo")
        wk_hi = wpool.tile([128, HW], F16, name="wk_hi")
        wq_lo = wpool.tile([128, HW], F16, name="wq_lo")
        wq_hi = wpool.tile([128, HW], F16, name="wq_hi")
        wv_t = wpool.tile([128, 2 * HW], F16, name="wv")
        wp_t = wpool.tile([128, KT * D], F16, name="wp")

        def xw_slice(lo, hi, dt, c0, c1):
            t = lo if dt < 8 else hi
            base = (dt % 8) * CH
            return t[:, base + c0:base + c1]

        def split8(ap):
            return ap.rearrange("p (t c) -> p t c", t=8)

        nc.sync.dma_start(split8(wk_lo[:]), wkv[:, 0:8, :])
        xc_t = {}

        def x_dma(ci, half, dst):
            # one DMA covering 8 d-tiles x CH tokens of chunk ci
            src = xT.rearrange("(t p) s -> p t s", p=128)
            nc.sync.dma_start(
                split8(dst[:]),
                src[:, 8 * half:8 * half + 8, ci * CH:(ci + 1) * CH])

        x0lo = xpool.tile([128, HW], F16, tag="xlo", name="xlo0")
        x_dma(0, 0, x0lo)
        nc.sync.dma_start(split8(wk_hi[:]), wkv[:, 8:16, :])
        x0hi = xpool.tile([128, HW], F16, tag="xhi", name="xhi0")
        x_dma(0, 1, x0hi)
        xc_t[0] = (x0lo, x0hi)
        nc.sync.dma_start(split8(wq_lo[:]), wqv[:, 0:8, :])
        nc.sync.dma_start(split8(wq_hi[:]), wqv[:, 8:16, :])
        csk = cpool.tile([HD, S], F16, name="cosk")
        nc.sync.dma_start(csk[:], cosk)
        snk = cpool.tile([HD, S], F16, name="sink")
        nc.sync.dma_start(snk[:], sink)
        csq = cpool.tile([HD, S], F16, name="cosq")
        nc.sync.dma_start(csq[:], cosq)
        snq = cpool.tile([HD, S], F16, name="sinq")
        nc.sync.dma_start(snq[:], sinq)
        nc.sync.dma_start(
            wv_t[:].rearrange("p (t c) -> p t c", t=16), wvv)
        nc.sync.dma_start(
            wp_t[:].rearrange("p (k c) -> p k c", k=KT), wpv)

        k_t = [kres.tile([HD, S], F16, tag="k", name=f"k{h}")
               for h in range(HPC)]
        v_t = [vres.tile([128, EL], F16, tag="v", name=f"v{st}")
               for st in range(S // 128)]
        z_part = [dram.tile([D, CH], F16, tag=f"zp{ci}", name=f"zp{ci}")
                  for ci in range(NCH)]
        z_red = [dram.tile([EL, CH], F16, tag=f"zr{ci}", name=f"zr{ci}")
                 for ci in range(NCH)]

        def rope_qk(ps, cs, sn, out_tile, sl):
            """out = ps*cos + rotate_half(ps)*sin (sign folded into sin)."""
            pre = rpool.tile([128, CH], F16, tag="pre")
            nc.vector.tensor_copy(pre[:], ps[:])
            rot = rpool.tile([128, CH], F16, tag="rot")
            nc.gpsimd.dma_start(rot[0:64, :], pre[64:128, :])
            nc.gpsimd.dma_start(rot[64:128, :], pre[0:64, :])
            t1 = rpool.tile([128, CH], F16, tag="t1")
            nc.vector.tensor_mul(t1[:], pre[:], cs)
            t2 = rpool.tile([128, CH], F16, tag="t2")
            nc.vector.tensor_mul(t2[:], rot[:], sn)
            nc.vector.tensor_add(out_tile[:, sl], t1[:], t2[:])

        for ci in range(NCH):
            tsl = slice(ci * CH, (ci + 1) * CH)
            if ci > 0:
                xlo = xpool.tile([128, HW], F16, tag="xlo", name=f"xlo{ci}")
                x_dma(ci, 0, xlo)
                xhi = xpool.tile([128, HW], F16, tag="xhi", name=f"xhi{ci}")
                x_dma(ci, 1, xhi)
                xc_t[ci] = (xlo, xhi)
            xlo, xhi = xc_t[ci]

            # ---------------- K chunk (+RoPE) ----------------
            for h in range(HPC):
                ps = ps_mm.tile([HD, CH], F32)
                for dt in range(DT):
                    nc.tensor.matmul(
                        ps[:],
                        xw_slice(wk_lo, wk_hi, dt, h * HD, (h + 1) * HD),
                        xw_slice(xlo, xhi, dt, 0, CH),
                        start=(dt == 0), stop=(dt == DT - 1))
                rope_qk(ps, csk[:, tsl], snk[:, tsl], k_t[h], tsl)

            # ---------------- Q chunk (+RoPE) ----------------
            qc = []
            for h in range(HPC):
                ps = ps_mm.tile([HD, CH], F32)
                for dt in range(DT):
                    nc.tensor.matmul(
                        ps[:],
                        xw_slice(wq_lo, wq_hi, dt, h * HD, (h + 1) * HD),
                        xw_slice(xlo, xhi, dt, 0, CH),
                        start=(dt == 0), stop=(dt == DT - 1))
                qt = qpool.tile([HD, CH], F16, tag="q", name=f"q{ci}_{h}")
                rope_qk(ps, csq[:, tsl], snq[:, tsl], qt, slice(None))
                qc.append(qt)

            # ---------------- V chunk (token-major) ----------------
            for st in range(CH // 128):
                ps = ps_mm.tile([128, EL], F32)
                for dt in range(DT):
                    nc.tensor.matmul(
                        ps[:],
                        xw_slice(xlo, xhi, dt, st * 128, (st + 1) * 128),
                        wv_t[:, dt * EL:(dt + 1) * EL],
                        start=(dt == 0), stop=(dt == DT - 1))
                nc.scalar.copy(v_t[ci * 4 + st][:], ps[:])

            # ---------------- causal attention for this chunk ----------------
            y_sb = []
            n_jt = 4 * ci + 4
            for h in range(HPC):
                o_ps = ps_o.tile([HD, CH], F32)
                r_ps = ps_r.tile([1, CH], F32)
                for jt in range(n_jt):
                    diag = jt - 4 * ci
                    off = 128 * diag if diag > 0 else 0
                    s_ps = ps_sc.tile([128, CH], F32)
                    nc.tensor.matmul(
                        s_ps[:, off:], k_t[h][:, jt * 128:(jt + 1) * 128],
                        qc[h][:, off:], start=True, stop=True)
                    p = ppool.tile([128, CH], F16, tag="p")
                    nc.scalar.activation(p[:, off:], s_ps[:, off:], AF.Exp)
                    if diag >= 0:
                        nc.vector.tensor_mul(
                            p[:, off:off + 128], p[:, off:off + 128], tri_t[:])
                    nc.tensor.matmul(
                        o_ps[:, off:], v_t[jt][:, h * HD:(h + 1) * HD],
                        p[:, off:], start=(jt == 0), stop=(jt == n_jt - 1))
                    nc.tensor.matmul(
                        r_ps[:, off:], ones_t[:], p[:, off:],
                        start=(jt == 0), stop=(jt == n_jt - 1))
                rinv = spool.tile([1, CH], F16, tag="rinv")
                nc.vector.reciprocal(rinv[:], r_ps[:])
                bcast = spool.tile([128, CH], F16, tag="bcast")
                nc.gpsimd.partition_broadcast(bcast[:], rinv[:], channels=128)
                ot = opool.tile([HD, CH], F16, tag="o", name=f"o{ci}_{h}")
                nc.vector.tensor_copy(ot[:], o_ps[:])
                yt = ypool.tile([HD, CH], F16, tag="y", name=f"y{ci}_{h}")
                nc.vector.tensor_mul(yt[:], ot[:], bcast[:])
                y_sb.append(yt)

            # ------- partial projection: zp[ci] = Wp[g-rows]^T @ y -------
            for half in range(2):
                zb = zpool.tile([128, 8 * CH], F16, tag="zb")
                for dd in range(8):
                    d2 = 8 * half + dd
                    ps = ps_mm.tile([128, CH], F32)
                    for kk in range(KT):
                        nc.tensor.matmul(
                            ps[:], wp_t[:, kk * D + d2 * 128:kk * D + (d2 + 1) * 128],
                            y_sb[kk][:], start=(kk == 0), stop=(kk == KT - 1))
                    nc.scalar.copy(zb[:, dd * CH:(dd + 1) * CH], ps[:])
                dst = z_part[ci][half * 8 * 128:(half + 1) * 8 * 128, :]
                nc.scalar.dma_start(
                    dst.rearrange("(t p) c -> p t c", p=128), split8(zb[:]))

            # ------- ReduceScatter: sum partials, keep own 512-dim slice ----
            nc.gpsimd.collective_compute(
                "ReduceScatter", mybir.AluOpType.add,
                replica_groups=GROUPS,
                ins=[z_part[ci].opt()], outs=[z_red[ci].opt()])

        for ci in range(NCH):
            nc.sync.dma_start(zc[ci], z_red[ci][:])
    nc.compile()
    return nc


def _tables():
    inv_freq = 1.0 / (ROPE_THETA ** (np.arange(0, HD, 2, dtype=np.float64) / HD))
    pos = np.arange(S, dtype=np.float64)
    f_half = np.outer(inv_freq, pos)                  # [64, S]
    freqs = np.concatenate([f_half, f_half], axis=0)  # [HD, S]
    # match reference numerics: cos/sin computed in float32 domain
    emb32 = freqs.astype(np.float32)
    cos_t = np.cos(emb32)
    sin_t = np.sin(emb32)
    scale = np.float32(HD ** -0.5)
    sgn = np.where(np.arange(HD) < HD // 2, -1.0, 1.0).astype(np.float32)[:, None]
    cosq = (cos_t * scale).astype(np.float16)
    sinq = (sin_t * sgn * scale).astype(np.float16)
    cosk = cos_t.astype(np.float16)
    sink = (sin_t * sgn).astype(np.float16)
    return cosq, sinq, cosk, sink


_NC_CACHE = {}


def _get_nc():
    if "nc" not in _NC_CACHE:
        _NC_CACHE["nc"] = _build()
    return _NC_CACHE["nc"]


def make_in_maps(x, W_attn, W_proj):
    x = np.asarray(x, dtype=np.float32)
    W_attn = np.asarray(W_attn, dtype=np.float32)
    W_proj = np.asarray(W_proj, dtype=np.float32)
    cosq, sinq, cosk, sink = _tables()
    tri = np.triu(np.ones((128, 128), np.float16))   # [jj, ii]: keep jj <= ii
    ones = np.ones((128, 1), np.float16)
    in_maps = []
    for c in range(N_CORES):
        b, g = divmod(c, HPC)
        in_maps.append({
            "xT": np.ascontiguousarray(x[b].T).astype(np.float16),
            "wq": W_attn[:, g * EL:(g + 1) * EL].astype(np.float16),
            "wk": W_attn[:, D + g * EL:D + (g + 1) * EL].astype(np.float16),
            "wv": W_attn[:, 2 * D + g * EL:2 * D + (g + 1) * EL].astype(np.float16),
            "wp": W_proj[g * EL:(g + 1) * EL, :].astype(np.float16),
            "cosq": cosq, "sinq": sinq, "cosk": cosk, "sink": sink,
            "tri": tri, "ones": ones,
        })
    return in_maps


def assemble(results):
    out = np.empty((B, S, D), dtype=np.float32)
    for c in range(N_CORES):
        b, g = divmod(c, HPC)
        zcs = np.asarray(results[c]["zc"], dtype=np.float32)
        for ci in range(NCH):
            out[b, ci * CH:(ci + 1) * CH, g * EL:(g + 1) * EL] = zcs[ci].T
    return out


def kernel(x, W_attn, W_proj):
    nc = _get_nc()
    in_maps = make_in_maps(x, W_attn, W_proj)
    res = bass_utils.run_bass_kernel_spmd(
        nc, in_maps, core_ids=list(range(N_CORES)), trace=False)
    return assemble(res.results)


if __name__ == "__main__":
    rng = np.random.default_rng(0)
    x = rng.standard_normal((B, S, D)).astype(np.float32)
    W_attn = (rng.standard_normal((D, 3 * D)) * D ** -0.5).astype(np.float32)
    W_proj = (rng.standard_normal((D, D)) * D ** -0.5).astype(np.float32)
    out = kernel(x, W_attn, W_proj)
    print("out", out.shape, out.dtype, np.abs(out).mean())


# revision 22
# speedup vs baseline: 2.0995x; 1.0038x over previous
"""Causal self-attention with RoPE on 8 TRN2 NeuronCores.

Sharding: core c -> (batch b = c//4, head-group g = c%4; 4 heads of 128 each).
Tensor-parallel over heads x data-parallel over batch.

Single fused pipeline per core, one pass over x. For each 512-token chunk ci:
  QKV matmuls (+RoPE on Q,K) -> causal attention over kv-chunks <= ci
  -> project own heads through own W_proj row-block into a full-D partial z
  -> per-chunk ReduceScatter(add) within the batch group produces this core's
     final 512-dim slice of z^T. Only the last chunk's collective sits on the
     critical path (~28us) vs. the serial AllGather tail it replaces.

All matmul operands are fp16 (1 PE cycle/row, same as bf16, 10-bit mantissa).
PSUM accumulation and softmax statistics stay fp32.

DMA queues are specialized to avoid head-of-line blocking of the input
prefetch stream: sync = input loads only (big batched transfers, halved so
the first matmuls of a chain can start early), DVE = RoPE rotate-half
SBUF-to-SBUF swaps, Act = z-partial spills (their producer is the Act copy),
Pool = collectives. The 1/rowsum broadcast runs on Pool's partition_broadcast
instead of a PE matmul.

Layouts (no on-chip transposes needed):
  xT   [D, S]   = x[b].T                      (host-transposed)
  Q^T,K^T [128, S] per head  (from matmul: lhsT=W-block, rhs=xT)
  V    [S, 512] token-major  (from matmul: lhsT=xT-tile, rhs=Wv)
  S^T  [j, i] scores blocks -> softmax sums via ones-matmul on PE
  O^T  [c, i] accumulated in PSUM, normalized by 1/rowsum afterwards
  zp   [D, S-chunk] partial projection, ReduceScatter -> z_red [512, 512]
"""
from contextlib import ExitStack

import numpy as np

import concourse.bass as bass
import concourse.tile as tile
import concourse.mybir as mybir
from concourse import bacc, bass_utils

B = 2
S = 2048
D = 2048
NH, HD = 16, 128
HPC = 4                 # heads per core
EL = HPC * HD           # 512: local e-width per core
CH = 512                # token-chunk width
NCH = S // CH           # 4
DT = D // 128           # 16 d-tiles
KT = EL // 128          # 4 k-tiles of the local proj contraction
ROPE_THETA = 10000.0
N_CORES = 8
GROUPS = [[0, 1, 2, 3], [4, 5, 6, 7]]

F32 = mybir.dt.float32
F16 = mybir.dt.float16
AF = mybir.ActivationFunctionType


def _build():
    nc = bacc.Bacc("TRN2", target_bir_lowering=False, debug=False,
                   enable_asserts=True, num_devices=N_CORES)
    xT = nc.dram_tensor("xT", [D, S], F16, kind="ExternalInput").ap()
    wq = nc.dram_tensor("wq", [D, EL], F16, kind="ExternalInput").ap()
    wk = nc.dram_tensor("wk", [D, EL], F16, kind="ExternalInput").ap()
    wv = nc.dram_tensor("wv", [D, EL], F16, kind="ExternalInput").ap()
    wp = nc.dram_tensor("wp", [EL, D], F16, kind="ExternalInput").ap()
    cosq = nc.dram_tensor("cosq", [HD, S], F16, kind="ExternalInput").ap()
    sinq = nc.dram_tensor("sinq", [HD, S], F16, kind="ExternalInput").ap()
    cosk = nc.dram_tensor("cosk", [HD, S], F16, kind="ExternalInput").ap()
    sink = nc.dram_tensor("sink", [HD, S], F16, kind="ExternalInput").ap()
    tri = nc.dram_tensor("tri", [128, 128], F16, kind="ExternalInput").ap()
    ones = nc.dram_tensor("ones", [128, 1], F16, kind="ExternalInput").ap()
    zc = nc.dram_tensor("zc", [NCH, EL, CH], F16, kind="ExternalOutput").ap()

    # [p, t, e] views: 128 partitions, d-tiles stacked along a middle dim
    wqv = wq.rearrange("(t p) e -> p t e", p=128)
    wkv = wk.rearrange("(t p) e -> p t e", p=128)
    wvv = wv.rearrange("(t p) e -> p t e", p=128)
    wpv = wp.rearrange("(k p) d -> p k d", p=128)

    HW = 8 * CH          # half-width of a batched x / qkv-weight transfer

    with tile.TileContext(nc) as tc, \
         nc.allow_low_precision(reason="fp16 attention"), ExitStack() as ctx:
        cpool = ctx.enter_context(tc.tile_pool(name="const", bufs=1))
        wpool = ctx.enter_context(tc.tile_pool(name="wts", bufs=1))
        xpool = ctx.enter_context(tc.tile_pool(name="xc", bufs=2))
        kres = ctx.enter_context(tc.tile_pool(name="kres", bufs=4))
        vres = ctx.enter_context(tc.tile_pool(name="vres", bufs=16))
        qpool = ctx.enter_context(tc.tile_pool(name="qp", bufs=5))
        prepool = ctx.enter_context(tc.tile_pool(name="pre", bufs=8))
        rpool = ctx.enter_context(tc.tile_pool(name="rope", bufs=2))
        ppool = ctx.enter_context(tc.tile_pool(name="pp", bufs=4))
        opool = ctx.enter_context(tc.tile_pool(name="op", bufs=4))
        spool = ctx.enter_context(tc.tile_pool(name="sm", bufs=3))
        ypool = ctx.enter_context(tc.tile_pool(name="yp", bufs=5))
        zpool = ctx.enter_context(tc.tile_pool(name="zp", bufs=2))
        dram = ctx.enter_context(tc.tile_pool(name="dram", bufs=1, space="DRAM"))
        ps_mm = ctx.enter_context(tc.tile_pool(name="ps_mm", bufs=2, space="PSUM"))
        ps_sc = ctx.enter_context(tc.tile_pool(name="ps_sc", bufs=3, space="PSUM"))
        ps_o = ctx.enter_context(tc.tile_pool(name="ps_o", bufs=2, space="PSUM"))
        ps_r = ctx.enter_context(tc.tile_pool(name="ps_r", bufs=1, space="PSUM"))

        # ---- prefetch stream (sync queue order == arrival priority) ----
        tri_t = cpool.tile([128, 128], F16)
        nc.sync.dma_start(tri_t[:], tri)
        ones_t = cpool.tile([128, 1], F16)
        nc.sync.dma_start(ones_t[:], ones)

        wk_lo = wpool.tile([128, HW], F16, name="wk_lo")
        wk_hi = wpool.tile([128, HW], F16, name="wk_hi")
        wq_lo = wpool.tile([128, HW], F16, name="wq_lo")
        wq_hi = wpool.tile([128, HW], F16, name="wq_hi")
        wv_t = wpool.tile([128, 2 * HW], F16, name="wv")
        wp_t = wpool.tile([128, KT * D], F16, name="wp")

        def xw_slice(lo, hi, dt, c0, c1):
            t = lo if dt < 8 else hi
            base = (dt % 8) * CH
            return t[:, base + c0:base + c1]

        def split8(ap):
            return ap.rearrange("p (t c) -> p t c", t=8)

        nc.sync.dma_start(split8(wk_lo[:]), wkv[:, 0:8, :])
        xc_t = {}

        def x_dma(ci, half, dst):
            # one DMA covering 8 d-tiles x CH tokens of chunk ci
            src = xT.rearrange("(t p) s -> p t s", p=128)
            nc.sync.dma_start(
                split8(dst[:]),
                src[:, 8 * half:8 * half + 8, ci * CH:(ci + 1) * CH])

        x0lo = xpool.tile([128, HW], F16, tag="xlo", name="xlo0")
        x_dma(0, 0, x0lo)
        nc.sync.dma_start(split8(wk_hi[:]), wkv[:, 8:16, :])
        x0hi = xpool.tile([128, HW], F16, tag="xhi", name="xhi0")
        x_dma(0, 1, x0hi)
        xc_t[0] = (x0lo, x0hi)
        nc.sync.dma_start(split8(wq_lo[:]), wqv[:, 0:8, :])
        nc.sync.dma_start(split8(wq_hi[:]), wqv[:, 8:16, :])
        csk = cpool.tile([HD, S], F16, name="cosk")
        nc.sync.dma_start(csk[:], cosk)
        snk = cpool.tile([HD, S], F16, name="sink")
        nc.sync.dma_start(snk[:], sink)
        csq = cpool.tile([HD, S], F16, name="cosq")
        nc.sync.dma_start(csq[:], cosq)
        snq = cpool.tile([HD, S], F16, name="sinq")
        nc.sync.dma_start(snq[:], sinq)
        nc.sync.dma_start(
            wv_t[:].rearrange("p (t c) -> p t c", t=16), wvv)
        nc.sync.dma_start(
            wp_t[:].rearrange("p (k c) -> p k c", k=KT), wpv)

        k_t = [kres.tile([HD, S], F16, tag="k", name=f"k{h}")
               for h in range(HPC)]
        v_t = [vres.tile([128, EL], F16, tag="v", name=f"v{st}")
               for st in range(S // 128)]
        z_part = [dram.tile([D, CH], F16, tag=f"zp{ci}", name=f"zp{ci}")
                  for ci in range(NCH)]
        z_red = [dram.tile([EL, CH], F16, tag=f"zr{ci}", name=f"zr{ci}")
                 for ci in range(NCH)]

        def rope_pre(ps):
            """Phase A: drain the QKV PSUM chain to SBUF (frees the bank so
            the next PE chain never waits on downstream RoPE progress)."""
            pre = prepool.tile([128, CH], F16, tag="pre")
            nc.vector.tensor_copy(pre[:], ps[:])
            return pre

        def rope_apply(pre, cs, sn, out_tile, sl):
            """Phase B: out = pre*cos + rotate_half(pre)*sin (sign in sin)."""
            rot = rpool.tile([128, CH], F16, tag="rot")
            nc.gpsimd.dma_start(rot[0:64, :], pre[64:128, :])
            nc.gpsimd.dma_start(rot[64:128, :], pre[0:64, :])
            t1 = rpool.tile([128, CH], F16, tag="t1")
            nc.vector.tensor_mul(t1[:], pre[:], cs)
            t2 = rpool.tile([128, CH], F16, tag="t2")
            nc.vector.tensor_mul(t2[:], rot[:], sn)
            nc.vector.tensor_add(out_tile[:, sl], t1[:], t2[:])

        for ci in range(NCH):
            tsl = slice(ci * CH, (ci + 1) * CH)
            if ci > 0:
                xlo = xpool.tile([128, HW], F16, tag="xlo", name=f"xlo{ci}")
                x_dma(ci, 0, xlo)
                xhi = xpool.tile([128, HW], F16, tag="xhi", name=f"xhi{ci}")
                x_dma(ci, 1, xhi)
                xc_t[ci] = (xlo, xhi)
            xlo, xhi = xc_t[ci]

            # ---------------- K chunk (+RoPE) ----------------
            k_pre = []
            for h in range(HPC):
                ps = ps_mm.tile([HD, CH], F32)
                for dt in range(DT):
                    nc.tensor.matmul(
                        ps[:],
                        xw_slice(wk_lo, wk_hi, dt, h * HD, (h + 1) * HD),
                        xw_slice(xlo, xhi, dt, 0, CH),
                        start=(dt == 0), stop=(dt == DT - 1))
                k_pre.append(rope_pre(ps))

            # ---------------- Q chunk (+RoPE) ----------------
            qc = []
            q_pre = []
            for h in range(HPC):
                ps = ps_mm.tile([HD, CH], F32)
                for dt in range(DT):
                    nc.tensor.matmul(
                        ps[:],
                        xw_slice(wq_lo, wq_hi, dt, h * HD, (h + 1) * HD),
                        xw_slice(xlo, xhi, dt, 0, CH),
                        start=(dt == 0), stop=(dt == DT - 1))
                q_pre.append(rope_pre(ps))
                qc.append(qpool.tile([HD, CH], F16, tag="q", name=f"q{ci}_{h}"))
            for h in range(HPC):
                rope_apply(k_pre[h], csk[:, tsl], snk[:, tsl], k_t[h], tsl)
                rope_apply(q_pre[h], csq[:, tsl], snq[:, tsl], qc[h],
                           slice(None))

            # ---------------- V chunk (token-major) ----------------
            for st in range(CH // 128):
                ps = ps_mm.tile([128, EL], F32)
                for dt in range(DT):
                    nc.tensor.matmul(
                        ps[:],
                        xw_slice(xlo, xhi, dt, st * 128, (st + 1) * 128),
                        wv_t[:, dt * EL:(dt + 1) * EL],
                        start=(dt == 0), stop=(dt == DT - 1))
                nc.scalar.copy(v_t[ci * 4 + st][:], ps[:])

            # ---------------- causal attention for this chunk ----------------
            y_sb = []
            n_jt = 4 * ci + 4
            for h in range(HPC):
                o_ps = ps_o.tile([HD, CH], F32)
                r_ps = ps_r.tile([1, CH], F32)
                for jt in range(n_jt):
                    diag = jt - 4 * ci
                    off = 128 * diag if diag > 0 else 0
                    s_ps = ps_sc.tile([128, CH], F32)
                    nc.tensor.matmul(
                        s_ps[:, off:], k_t[h][:, jt * 128:(jt + 1) * 128],
                        qc[h][:, off:], start=True, stop=True)
                    p = ppool.tile([128, CH], F16, tag="p")
                    nc.scalar.activation(p[:, off:], s_ps[:, off:], AF.Exp)
                    if diag >= 0:
                        nc.vector.tensor_mul(
                            p[:, off:off + 128], p[:, off:off + 128], tri_t[:])
                    nc.tensor.matmul(
                        o_ps[:, off:], v_t[jt][:, h * HD:(h + 1) * HD],
                        p[:, off:], start=(jt == 0), stop=(jt == n_jt - 1))
                    nc.tensor.matmul(
                        r_ps[:, off:], ones_t[:], p[:, off:],
                        start=(jt == 0), stop=(jt == n_jt - 1))
                rinv = spool.tile([1, CH], F16, tag="rinv")
                nc.vector.reciprocal(rinv[:], r_ps[:])
                bcast = spool.tile([128, CH], F16, tag="bcast")
                nc.gpsimd.partition_broadcast(bcast[:], rinv[:], channels=128)
                ot = opool.tile([HD, CH], F16, tag="o", name=f"o{ci}_{h}")
                nc.vector.tensor_copy(ot[:], o_ps[:])
                yt = ypool.tile([HD, CH], F16, tag="y", name=f"y{ci}_{h}")
                nc.vector.tensor_mul(yt[:], ot[:], bcast[:])
                y_sb.append(yt)

            # ------- partial projection: zp[ci] = Wp[g-rows]^T @ y -------
            for half in range(2):
                zb = zpool.tile([128, 8 * CH], F16, tag="zb")
                for dd in range(8):
                    d2 = 8 * half + dd
                    ps = ps_mm.tile([128, CH], F32)
                    for kk in range(KT):
                        nc.tensor.matmul(
                            ps[:], wp_t[:, kk * D + d2 * 128:kk * D + (d2 + 1) * 128],
                            y_sb[kk][:], start=(kk == 0), stop=(kk == KT - 1))
                    nc.scalar.copy(zb[:, dd * CH:(dd + 1) * CH], ps[:])
                dst = z_part[ci][half * 8 * 128:(half + 1) * 8 * 128, :]
                nc.scalar.dma_start(
                    dst.rearrange("(t p) c -> p t c", p=128), split8(zb[:]))

            # ------- ReduceScatter: sum partials, keep own 512-dim slice ----
            nc.gpsimd.collective_compute(
                "ReduceScatter", mybir.AluOpType.add,
                replica_groups=GROUPS,
                ins=[z_part[ci].opt()], outs=[z_red[ci].opt()])

        for ci in range(NCH):
            nc.sync.dma_start(zc[ci], z_red[ci][:])
    nc.compile()
    return nc


def _tables():
    inv_freq = 1.0 / (ROPE_THETA ** (np.arange(0, HD, 2, dtype=np.float64) / HD))
    pos = np.arange(S, dtype=np.float64)
    f_half = np.outer(inv_freq, pos)                  # [64, S]
    freqs = np.concatenate([f_half, f_half], axis=0)  # [HD, S]
    # match reference numerics: cos/sin computed in float32 domain
    emb32 = freqs.astype(np.float32)
    cos_t = np.cos(emb32)
    sin_t = np.sin(emb32)
    scale = np.float32(HD ** -0.5)
    sgn = np.where(np.arange(HD) < HD // 2, -1.0, 1.0).astype(np.float32)[:, None]
    cosq = (cos_t * scale).astype(np.float16)
    sinq = (sin_t * sgn * scale).astype(np.float16)
    cosk = cos_t.astype(np.float16)
    sink = (sin_t * sgn).astype(np.float16)
    return cosq, sinq, cosk, sink


_NC_CACHE = {}


def _get_nc():
    if "nc" not in _NC_CACHE:
        _NC_CACHE["nc"] = _build()
    return _NC_CACHE["nc"]


def make_in_maps(x, W_attn, W_proj):
    x = np.asarray(x, dtype=np.float32)
    W_attn = np.asarray(W_attn, dtype=np.float32)
    W_proj = np.asarray(W_proj, dtype=np.float32)
    cosq, sinq, cosk, sink = _tables()
    tri = np.triu(np.ones((128, 128), np.float16))   # [jj, ii]: keep jj <= ii
    ones = np.ones((128, 1), np.float16)
    in_maps = []
    for c in range(N_CORES):
        b, g = divmod(c, HPC)
        in_maps.append({
            "xT": np.ascontiguousarray(x[b].T).astype(np.float16),
            "wq": W_attn[:, g * EL:(g + 1) * EL].astype(np.float16),
            "wk": W_attn[:, D + g * EL:D + (g + 1) * EL].astype(np.float16),
            "wv": W_attn[:, 2 * D + g * EL:2 * D + (g + 1) * EL].astype(np.float16),
            "wp": W_proj[g * EL:(g + 1) * EL, :].astype(np.float16),
            "cosq": cosq, "sinq": sinq, "cosk": cosk, "sink": sink,
            "tri": tri, "ones": ones,
        })
    return in_maps


def assemble(results):
    out = np.empty((B, S, D), dtype=np.float32)
    for c in range(N_CORES):
        b, g = divmod(c, HPC)
        zcs = np.asarray(results[c]["zc"], dtype=np.float32)
        for ci in range(NCH):
            out[b, ci * CH:(ci + 1) * CH, g * EL:(g + 1) * EL] = zcs[ci].T
    return out


def kernel(x, W_attn, W_proj):
    nc = _get_nc()
    in_maps = make_in_maps(x, W_attn, W_proj)
    res = bass_utils.run_bass_kernel_spmd(
        nc, in_maps, core_ids=list(range(N_CORES)), trace=False)
    return assemble(res.results)


if __name__ == "__main__":
    rng = np.random.default_rng(0)
    x = rng.standard_normal((B, S, D)).astype(np.float32)
    W_attn = (rng.standard_normal((D, 3 * D)) * D ** -0.5).astype(np.float32)
    W_proj = (rng.standard_normal((D, D)) * D ** -0.5).astype(np.float32)
    out = kernel(x, W_attn, W_proj)
    print("out", out.shape, out.dtype, np.abs(out).mean())


# revision 25
# speedup vs baseline: 2.1745x; 1.0357x over previous
"""Causal self-attention with RoPE on 8 TRN2 NeuronCores.

Sharding: core c -> (batch b = c//4, head-group g = c%4; 4 heads of 128 each).
Tensor-parallel over heads x data-parallel over batch.

Single fused pipeline per core, one pass over x. For each 512-token chunk ci:
  QKV matmuls (+RoPE on Q,K) -> causal attention over kv-chunks <= ci
  -> project own heads through own W_proj row-block into a full-D partial z
  -> per-chunk ReduceScatter(add) within the batch group produces this core's
     final 512-dim slice of z^T. Only the last chunk's collective sits on the
     critical path (~28us) vs. the serial AllGather tail it replaces.

All matmul operands are fp16 (1 PE cycle/row, same as bf16, 10-bit mantissa).
PSUM accumulation and softmax statistics stay fp32.

DMA queues are specialized to avoid head-of-line blocking of the input
prefetch stream: sync = input loads only (big batched transfers, halved so
the first matmuls of a chain can start early), DVE = RoPE rotate-half
SBUF-to-SBUF swaps, Act = z-partial spills (their producer is the Act copy),
Pool = collectives. The 1/rowsum broadcast runs on Pool's partition_broadcast
instead of a PE matmul.

Layouts (no on-chip transposes needed):
  xT   [D, S]   = x[b].T                      (host-transposed)
  Q^T,K^T [128, S] per head  (from matmul: lhsT=W-block, rhs=xT)
  V    [S, 512] token-major  (from matmul: lhsT=xT-tile, rhs=Wv)
  S^T  [j, i] scores blocks -> softmax sums via ones-matmul on PE
  O^T  [c, i] accumulated in PSUM, normalized by 1/rowsum afterwards
  zp   [D, S-chunk] partial projection, ReduceScatter -> z_red [512, 512]
"""
from contextlib import ExitStack

import numpy as np

import concourse.bass as bass
import concourse.tile as tile
import concourse.mybir as mybir
from concourse import bacc, bass_utils

B = 2
S = 2048
D = 2048
NH, HD = 16, 128
HPC = 4                 # heads per core
EL = HPC * HD           # 512: local e-width per core
CH = 512                # token-chunk width
NCH = S // CH           # 4
DT = D // 128           # 16 d-tiles
KT = EL // 128          # 4 k-tiles of the local proj contraction
ROPE_THETA = 10000.0
N_CORES = 8
GROUPS = [[0, 1, 2, 3], [4, 5, 6, 7]]

F32 = mybir.dt.float32
F16 = mybir.dt.float16
AF = mybir.ActivationFunctionType


def _build():
    nc = bacc.Bacc("TRN2", target_bir_lowering=False, debug=False,
                   enable_asserts=True, num_devices=N_CORES)
    xT = nc.dram_tensor("xT", [D, S], F16, kind="ExternalInput").ap()
    wq = nc.dram_tensor("wq", [D, EL], F16, kind="ExternalInput").ap()
    wk = nc.dram_tensor("wk", [D, EL], F16, kind="ExternalInput").ap()
    wv = nc.dram_tensor("wv", [D, EL], F16, kind="ExternalInput").ap()
    wp = nc.dram_tensor("wp", [EL, D], F16, kind="ExternalInput").ap()
    cosq = nc.dram_tensor("cosq", [HD, S], F16, kind="ExternalInput").ap()
    sinq = nc.dram_tensor("sinq", [HD, S], F16, kind="ExternalInput").ap()
    cosk = nc.dram_tensor("cosk", [HD, S], F16, kind="ExternalInput").ap()
    sink = nc.dram_tensor("sink", [HD, S], F16, kind="ExternalInput").ap()
    tri = nc.dram_tensor("tri", [128, 128], F16, kind="ExternalInput").ap()
    ones = nc.dram_tensor("ones", [128, 1], F16, kind="ExternalInput").ap()
    zc = nc.dram_tensor("zc", [NCH, EL, CH], F16, kind="ExternalOutput").ap()

    # [p, t, e] views: 128 partitions, d-tiles stacked along a middle dim
    wqv = wq.rearrange("(t p) e -> p t e", p=128)
    wkv = wk.rearrange("(t p) e -> p t e", p=128)
    wvv = wv.rearrange("(t p) e -> p t e", p=128)
    wpv = wp.rearrange("(k p) d -> p k d", p=128)

    HW = 8 * CH          # half-width of a batched x / qkv-weight transfer

    with tile.TileContext(nc) as tc, \
         nc.allow_low_precision(reason="fp16 attention"), ExitStack() as ctx:
        cpool = ctx.enter_context(tc.tile_pool(name="const", bufs=1))
        wpool = ctx.enter_context(tc.tile_pool(name="wts", bufs=1))
        xpool = ctx.enter_context(tc.tile_pool(name="xc", bufs=2))
        kres = ctx.enter_context(tc.tile_pool(name="kres", bufs=4))
        vres = ctx.enter_context(tc.tile_pool(name="vres", bufs=16))
        qpool = ctx.enter_context(tc.tile_pool(name="qp", bufs=5))
        prepool = ctx.enter_context(tc.tile_pool(name="pre", bufs=8))
        rpool = ctx.enter_context(tc.tile_pool(name="rope", bufs=2))
        ppool = ctx.enter_context(tc.tile_pool(name="pp", bufs=4))
        opool = ctx.enter_context(tc.tile_pool(name="op", bufs=4))
        spool = ctx.enter_context(tc.tile_pool(name="sm", bufs=3))
        ypool = ctx.enter_context(tc.tile_pool(name="yp", bufs=5))
        zpool = ctx.enter_context(tc.tile_pool(name="zp", bufs=2))
        dram = ctx.enter_context(tc.tile_pool(name="dram", bufs=1, space="DRAM"))
        ps_mm = ctx.enter_context(tc.tile_pool(name="ps_mm", bufs=2, space="PSUM"))
        ps_sc = ctx.enter_context(tc.tile_pool(name="ps_sc", bufs=3, space="PSUM"))
        ps_o = ctx.enter_context(tc.tile_pool(name="ps_o", bufs=2, space="PSUM"))
        ps_r = ctx.enter_context(tc.tile_pool(name="ps_r", bufs=1, space="PSUM"))

        # ---- prefetch stream (sync queue order == arrival priority) ----
        tri_t = cpool.tile([128, 128], F16)
        nc.sync.dma_start(tri_t[:], tri)
        ones_t = cpool.tile([128, 1], F16)
        nc.sync.dma_start(ones_t[:], ones)

        wk_lo = wpool.tile([128, HW], F16, name="wk_lo")
        wk_hi = wpool.tile([128, HW], F16, name="wk_hi")
        wq_lo = wpool.tile([128, HW], F16, name="wq_lo")
        wq_hi = wpool.tile([128, HW], F16, name="wq_hi")
        wv_t = wpool.tile([128, 2 * HW], F16, name="wv")
        wp_t = wpool.tile([128, KT * D], F16, name="wp")

        def xw_slice(lo, hi, dt, c0, c1):
            t = lo if dt < 8 else hi
            base = (dt % 8) * CH
            return t[:, base + c0:base + c1]

        def split8(ap):
            return ap.rearrange("p (t c) -> p t c", t=8)

        nc.sync.dma_start(split8(wk_lo[:]), wkv[:, 0:8, :])
        xc_t = {}

        def x_dma(ci, half, dst):
            # one DMA covering 8 d-tiles x CH tokens of chunk ci
            src = xT.rearrange("(t p) s -> p t s", p=128)
            nc.sync.dma_start(
                split8(dst[:]),
                src[:, 8 * half:8 * half + 8, ci * CH:(ci + 1) * CH])

        x0lo = xpool.tile([128, HW], F16, tag="xlo", name="xlo0")
        x_dma(0, 0, x0lo)
        nc.sync.dma_start(split8(wk_hi[:]), wkv[:, 8:16, :])
        x0hi = xpool.tile([128, HW], F16, tag="xhi", name="xhi0")
        x_dma(0, 1, x0hi)
        xc_t[0] = (x0lo, x0hi)
        nc.sync.dma_start(split8(wq_lo[:]), wqv[:, 0:8, :])
        nc.sync.dma_start(split8(wq_hi[:]), wqv[:, 8:16, :])
        nc.sync.dma_start(
            wv_t[:].rearrange("p (t c) -> p t c", t=16), wvv)
        csk = cpool.tile([HD, S], F16, name="cosk")
        nc.sync.dma_start(csk[:], cosk)
        snk = cpool.tile([HD, S], F16, name="sink")
        nc.sync.dma_start(snk[:], sink)
        csq = cpool.tile([HD, S], F16, name="cosq")
        nc.sync.dma_start(csq[:], cosq)
        snq = cpool.tile([HD, S], F16, name="sinq")
        nc.sync.dma_start(snq[:], sinq)

        k_t = [kres.tile([HD, S], F16, tag="k", name=f"k{h}")
               for h in range(HPC)]
        v_t = [vres.tile([128, EL], F16, tag="v", name=f"v{st}")
               for st in range(S // 128)]
        z_part = [dram.tile([D, CH], F16, tag=f"zp{ci}", name=f"zp{ci}")
                  for ci in range(NCH)]
        z_red = [dram.tile([EL, CH], F16, tag=f"zr{ci}", name=f"zr{ci}")
                 for ci in range(NCH)]

        def rope_pre(ps):
            """Phase A: drain the QKV PSUM chain to SBUF (frees the bank so
            the next PE chain never waits on downstream RoPE progress)."""
            pre = prepool.tile([128, CH], F16, tag="pre")
            nc.scalar.copy(pre[:], ps[:])
            return pre

        def rope_apply(pre, cs, sn, out_tile, sl):
            """Phase B: out = pre*cos + rotate_half(pre)*sin (sign in sin)."""
            rot = rpool.tile([128, CH], F16, tag="rot")
            nc.gpsimd.dma_start(rot[0:64, :], pre[64:128, :])
            nc.gpsimd.dma_start(rot[64:128, :], pre[0:64, :])
            t1 = rpool.tile([128, CH], F16, tag="t1")
            nc.vector.tensor_mul(t1[:], pre[:], cs)
            t2 = rpool.tile([128, CH], F16, tag="t2")
            nc.vector.tensor_mul(t2[:], rot[:], sn)
            nc.vector.tensor_add(out_tile[:, sl], t1[:], t2[:])

        for ci in range(NCH):
            tsl = slice(ci * CH, (ci + 1) * CH)
            if ci + 1 < NCH:
                # prefetch next chunk's x ahead of the bulkier wp load so the
                # in-order DMA queue matches consumption order
                nlo = xpool.tile([128, HW], F16, tag="xlo", name=f"xlo{ci+1}")
                x_dma(ci + 1, 0, nlo)
                nhi = xpool.tile([128, HW], F16, tag="xhi", name=f"xhi{ci+1}")
                x_dma(ci + 1, 1, nhi)
                xc_t[ci + 1] = (nlo, nhi)
            if ci == 0:
                nc.sync.dma_start(
                    wp_t[:].rearrange("p (k c) -> p k c", k=KT), wpv)
            xlo, xhi = xc_t[ci]

            # ---------------- K chunk (+RoPE) ----------------
            k_pre = []
            for h in range(HPC):
                ps = ps_mm.tile([HD, CH], F32)
                for dt in range(DT):
                    nc.tensor.matmul(
                        ps[:],
                        xw_slice(wk_lo, wk_hi, dt, h * HD, (h + 1) * HD),
                        xw_slice(xlo, xhi, dt, 0, CH),
                        start=(dt == 0), stop=(dt == DT - 1))
                k_pre.append(rope_pre(ps))

            # ---------------- Q chunk (+RoPE) ----------------
            qc = []
            q_pre = []
            for h in range(HPC):
                ps = ps_mm.tile([HD, CH], F32)
                for dt in range(DT):
                    nc.tensor.matmul(
                        ps[:],
                        xw_slice(wq_lo, wq_hi, dt, h * HD, (h + 1) * HD),
                        xw_slice(xlo, xhi, dt, 0, CH),
                        start=(dt == 0), stop=(dt == DT - 1))
                q_pre.append(rope_pre(ps))
                qc.append(qpool.tile([HD, CH], F16, tag="q", name=f"q{ci}_{h}"))
            for h in range(HPC):
                rope_apply(k_pre[h], csk[:, tsl], snk[:, tsl], k_t[h], tsl)
                rope_apply(q_pre[h], csq[:, tsl], snq[:, tsl], qc[h],
                           slice(None))

            # ---------------- V chunk (token-major) ----------------
            for st in range(CH // 128):
                ps = ps_mm.tile([128, EL], F32)
                for dt in range(DT):
                    nc.tensor.matmul(
                        ps[:],
                        xw_slice(xlo, xhi, dt, st * 128, (st + 1) * 128),
                        wv_t[:, dt * EL:(dt + 1) * EL],
                        start=(dt == 0), stop=(dt == DT - 1))
                nc.scalar.copy(v_t[ci * 4 + st][:], ps[:])

            # ---------------- causal attention for this chunk ----------------
            y_sb = []
            n_jt = 4 * ci + 4
            for h in range(HPC):
                o_ps = ps_o.tile([HD, CH], F32)
                r_ps = ps_r.tile([1, CH], F32)
                for jt in range(n_jt):
                    diag = jt - 4 * ci
                    off = 128 * diag if diag > 0 else 0
                    s_ps = ps_sc.tile([128, CH], F32)
                    nc.tensor.matmul(
                        s_ps[:, off:], k_t[h][:, jt * 128:(jt + 1) * 128],
                        qc[h][:, off:], start=True, stop=True)
                    p = ppool.tile([128, CH], F16, tag="p")
                    nc.scalar.activation(p[:, off:], s_ps[:, off:], AF.Exp)
                    if diag >= 0:
                        nc.vector.tensor_mul(
                            p[:, off:off + 128], p[:, off:off + 128], tri_t[:])
                    nc.tensor.matmul(
                        o_ps[:, off:], v_t[jt][:, h * HD:(h + 1) * HD],
                        p[:, off:], start=(jt == 0), stop=(jt == n_jt - 1))
                    nc.tensor.matmul(
                        r_ps[:, off:], ones_t[:], p[:, off:],
                        start=(jt == 0), stop=(jt == n_jt - 1))
                rinv = spool.tile([1, CH], F16, tag="rinv")
                nc.vector.reciprocal(rinv[:], r_ps[:])
                bcast = spool.tile([128, CH], F16, tag="bcast")
                nc.gpsimd.partition_broadcast(bcast[:], rinv[:], channels=128)
                ot = opool.tile([HD, CH], F16, tag="o", name=f"o{ci}_{h}")
                nc.vector.tensor_copy(ot[:], o_ps[:])
                yt = ypool.tile([HD, CH], F16, tag="y", name=f"y{ci}_{h}")
                nc.vector.tensor_mul(yt[:], ot[:], bcast[:])
                y_sb.append(yt)

            # ------- partial projection: zp[ci] = Wp[g-rows]^T @ y -------
            for half in range(2):
                zb = zpool.tile([128, 8 * CH], F16, tag="zb")
                for dd in range(8):
                    d2 = 8 * half + dd
                    ps = ps_mm.tile([128, CH], F32)
                    for kk in range(KT):
                        nc.tensor.matmul(
                            ps[:], wp_t[:, kk * D + d2 * 128:kk * D + (d2 + 1) * 128],
                            y_sb[kk][:], start=(kk == 0), stop=(kk == KT - 1))
                    nc.scalar.copy(zb[:, dd * CH:(dd + 1) * CH], ps[:])
                dst = z_part[ci][half * 8 * 128:(half + 1) * 8 * 128, :]
                nc.scalar.dma_start(
                    dst.rearrange("(t p) c -> p t c", p=128), split8(zb[:]))

            # ------- ReduceScatter: sum partials, keep own 512-dim slice ----
            nc.gpsimd.collective_compute(
                "ReduceScatter", mybir.AluOpType.add,
                replica_groups=GROUPS,
                ins=[z_part[ci].opt()], outs=[z_red[ci].opt()])

        for ci in range(NCH):
            nc.sync.dma_start(zc[ci], z_red[ci][:])
    nc.compile()
    return nc


def _tables():
    inv_freq = 1.0 / (ROPE_THETA ** (np.arange(0, HD, 2, dtype=np.float64) / HD))
    pos = np.arange(S, dtype=np.float64)
    f_half = np.outer(inv_freq, pos)                  # [64, S]
    freqs = np.concatenate([f_half, f_half], axis=0)  # [HD, S]
    # match reference numerics: cos/sin computed in float32 domain
    emb32 = freqs.astype(np.float32)
    cos_t = np.cos(emb32)
    sin_t = np.sin(emb32)
    scale = np.float32(HD ** -0.5)
    sgn = np.where(np.arange(HD) < HD // 2, -1.0, 1.0).astype(np.float32)[:, None]
    cosq = (cos_t * scale).astype(np.float16)
    sinq = (sin_t * sgn * scale).astype(np.float16)
    cosk = cos_t.astype(np.float16)
    sink = (sin_t * sgn).astype(np.float16)
    return cosq, sinq, cosk, sink


_NC_CACHE = {}


def _get_nc():
    if "nc" not in _NC_CACHE:
        _NC_CACHE["nc"] = _build()
    return _NC_CACHE["nc"]


def make_in_maps(x, W_attn, W_proj):
    x = np.asarray(x, dtype=np.float32)
    W_attn = np.asarray(W_attn, dtype=np.float32)
    W_proj = np.asarray(W_proj, dtype=np.float32)
    cosq, sinq, cosk, sink = _tables()
    tri = np.triu(np.ones((128, 128), np.float16))   # [jj, ii]: keep jj <= ii
    ones = np.ones((128, 1), np.float16)
    in_maps = []
    for c in range(N_CORES):
        b, g = divmod(c, HPC)
        in_maps.append({
            "xT": np.ascontiguousarray(x[b].T).astype(np.float16),
            "wq": W_attn[:, g * EL:(g + 1) * EL].astype(np.float16),
            "wk": W_attn[:, D + g * EL:D + (g + 1) * EL].astype(np.float16),
            "wv": W_attn[:, 2 * D + g * EL:2 * D + (g + 1) * EL].astype(np.float16),
            "wp": W_proj[g * EL:(g + 1) * EL, :].astype(np.float16),
            "cosq": cosq, "sinq": sinq, "cosk": cosk, "sink": sink,
            "tri": tri, "ones": ones,
        })
    return in_maps


def assemble(results):
    out = np.empty((B, S, D), dtype=np.float32)
    for c in range(N_CORES):
        b, g = divmod(c, HPC)
        zcs = np.asarray(results[c]["zc"], dtype=np.float32)
        for ci in range(NCH):
            out[b, ci * CH:(ci + 1) * CH, g * EL:(g + 1) * EL] = zcs[ci].T
    return out


def kernel(x, W_attn, W_proj):
    nc = _get_nc()
    in_maps = make_in_maps(x, W_attn, W_proj)
    res = bass_utils.run_bass_kernel_spmd(
        nc, in_maps, core_ids=list(range(N_CORES)), trace=False)
    return assemble(res.results)


if __name__ == "__main__":
    rng = np.random.default_rng(0)
    x = rng.standard_normal((B, S, D)).astype(np.float32)
    W_attn = (rng.standard_normal((D, 3 * D)) * D ** -0.5).astype(np.float32)
    W_proj = (rng.standard_normal((D, D)) * D ** -0.5).astype(np.float32)
    out = kernel(x, W_attn, W_proj)
    print("out", out.shape, out.dtype, np.abs(out).mean())


# revision 27
# speedup vs baseline: 2.3474x; 1.0795x over previous
"""Causal self-attention with RoPE on 8 TRN2 NeuronCores.

Sharding: core c -> (batch b = c//4, head-group g = c%4; 4 heads of 128 each).
Tensor-parallel over heads x data-parallel over batch.

Single fused pipeline per core, one pass over x, token chunks of
[512, 512, 512, 256, 256]. Per chunk: QKV matmuls (+RoPE on Q,K) -> causal
attention over kv-blocks <= chunk end -> project own heads through own W_proj
row-block into a full-D partial z -> per-chunk ReduceScatter(add) within the
batch group produces this core's final 512-dim slice of z^T.

The projection for chunk i is EMITTED inside chunk i+1 (after its QKV
matmuls): the softmax-normalize latency chain (rowsum -> reciprocal ->
broadcast -> scale) then hides under 40us of next-chunk PE work instead of
stalling the in-order PE at every chunk boundary. The two final 256-token
chunks shrink the only exposed collective (last ReduceScatter: 21.5us vs
28us) and the tail projection.

All matmul operands are fp16 (1 PE cycle/row, same as bf16, 10-bit
mantissa). PSUM accumulation and softmax statistics stay fp32.

DMA queues: sync = input prefetch in consumption order + RoPE rotate-half
swaps; Act = z-partial spills (their producer is the Act PSUM-drain copy);
Pool = collectives + 1/rowsum partition_broadcast. Big input transfers are
batched (2 DMAs per x chunk / weight matrix) but halved so the first
accumulation chain can start after ~6us.

Layouts (no on-chip transposes needed):
  xT   [D, S]   = x[b].T                      (host-transposed)
  Q^T,K^T [128, S] per head  (from matmul: lhsT=W-block, rhs=xT)
  V    [S, 512] token-major  (from matmul: lhsT=xT-tile, rhs=Wv)
  S^T  [j, i] scores blocks -> softmax sums via ones-matmul on PE
  O^T  [c, i] accumulated in PSUM, normalized by 1/rowsum afterwards
  zp   [D, cw] partial projection, ReduceScatter -> z_red [512, cw] -> zT
"""
from contextlib import ExitStack

import numpy as np

import concourse.bass as bass
import concourse.tile as tile
import concourse.mybir as mybir
from concourse import bacc, bass_utils

B = 2
S = 2048
D = 2048
NH, HD = 16, 128
HPC = 4                 # heads per core
EL = HPC * HD           # 512: local e-width per core
CH = 512                # max token-chunk width
CHUNKS = [(0, 512), (512, 512), (1024, 512), (1536, 512)]
DT = D // 128           # 16 d-tiles
KT = EL // 128          # 4 k-tiles of the local proj contraction
ROPE_THETA = 10000.0
N_CORES = 8
GROUPS = [[0, 1, 2, 3], [4, 5, 6, 7]]

F32 = mybir.dt.float32
F16 = mybir.dt.float16
AF = mybir.ActivationFunctionType


def _build():
    nc = bacc.Bacc("TRN2", target_bir_lowering=False, debug=False,
                   enable_asserts=True, num_devices=N_CORES)
    xT = nc.dram_tensor("xT", [D, S], F16, kind="ExternalInput").ap()
    wq = nc.dram_tensor("wq", [D, EL], F16, kind="ExternalInput").ap()
    wk = nc.dram_tensor("wk", [D, EL], F16, kind="ExternalInput").ap()
    wv = nc.dram_tensor("wv", [D, EL], F16, kind="ExternalInput").ap()
    wp = nc.dram_tensor("wp", [EL, D], F16, kind="ExternalInput").ap()
    cosq = nc.dram_tensor("cosq", [HD, S], F16, kind="ExternalInput").ap()
    sinq = nc.dram_tensor("sinq", [HD, S], F16, kind="ExternalInput").ap()
    cosk = nc.dram_tensor("cosk", [HD, S], F16, kind="ExternalInput").ap()
    sink = nc.dram_tensor("sink", [HD, S], F16, kind="ExternalInput").ap()
    tri = nc.dram_tensor("tri", [128, 128], F16, kind="ExternalInput").ap()
    ones = nc.dram_tensor("ones", [128, 1], F16, kind="ExternalInput").ap()
    zc = nc.dram_tensor("zc", [EL, S], F16, kind="ExternalOutput").ap()

    # [p, t, e] views: 128 partitions, d-tiles stacked along a middle dim
    wqv = wq.rearrange("(t p) e -> p t e", p=128)
    wkv = wk.rearrange("(t p) e -> p t e", p=128)
    wvv = wv.rearrange("(t p) e -> p t e", p=128)
    wpv = wp.rearrange("(k p) d -> p k d", p=128)
    xTv = xT.rearrange("(t p) s -> p t s", p=128)

    HW = 8 * CH          # half-width of a batched x / qkv-weight transfer

    with tile.TileContext(nc) as tc, \
         nc.allow_low_precision(reason="fp16 attention"), ExitStack() as ctx:
        cpool = ctx.enter_context(tc.tile_pool(name="const", bufs=1))
        wpool = ctx.enter_context(tc.tile_pool(name="wts", bufs=1))
        xpool = ctx.enter_context(tc.tile_pool(name="xc", bufs=2))
        kres = ctx.enter_context(tc.tile_pool(name="kres", bufs=4))
        vres = ctx.enter_context(tc.tile_pool(name="vres", bufs=16))
        qpool = ctx.enter_context(tc.tile_pool(name="qp", bufs=5))
        prepool = ctx.enter_context(tc.tile_pool(name="pre", bufs=8))
        rpool = ctx.enter_context(tc.tile_pool(name="rope", bufs=2))
        ppool = ctx.enter_context(tc.tile_pool(name="pp", bufs=4))
        opool = ctx.enter_context(tc.tile_pool(name="op", bufs=4))
        spool = ctx.enter_context(tc.tile_pool(name="sm", bufs=3))
        ypool = ctx.enter_context(tc.tile_pool(name="yp", bufs=9))
        zpool = ctx.enter_context(tc.tile_pool(name="zp", bufs=2))
        dram = ctx.enter_context(tc.tile_pool(name="dram", bufs=1, space="DRAM"))
        ps_mm = ctx.enter_context(tc.tile_pool(name="ps_mm", bufs=2, space="PSUM"))
        ps_sc = ctx.enter_context(tc.tile_pool(name="ps_sc", bufs=3, space="PSUM"))
        ps_o = ctx.enter_context(tc.tile_pool(name="ps_o", bufs=2, space="PSUM"))
        ps_r = ctx.enter_context(tc.tile_pool(name="ps_r", bufs=1, space="PSUM"))

        # ---- prefetch stream (sync queue order == arrival priority) ----
        tri_t = cpool.tile([128, 128], F16)
        nc.sync.dma_start(tri_t[:], tri)
        ones_t = cpool.tile([128, 1], F16)
        nc.sync.dma_start(ones_t[:], ones)

        wk_lo = wpool.tile([128, HW], F16, name="wk_lo")
        wk_hi = wpool.tile([128, HW], F16, name="wk_hi")
        wq_lo = wpool.tile([128, HW], F16, name="wq_lo")
        wq_hi = wpool.tile([128, HW], F16, name="wq_hi")
        wv_t = wpool.tile([128, 2 * HW], F16, name="wv")
        wp_t = wpool.tile([128, KT * D], F16, name="wp")

        def xw_slice(lo, hi, dt, cw, c0, c1):
            t = lo if dt < 8 else hi
            base = (dt % 8) * cw
            return t[:, base + c0:base + c1]

        def split8(ap):
            return ap.rearrange("p (t c) -> p t c", t=8)

        nc.sync.dma_start(split8(wk_lo[:]), wkv[:, 0:8, :])
        xc_t = {}

        def x_dma(ci, half, dst):
            # one DMA covering 8 d-tiles x cw tokens of chunk ci
            start, cw = CHUNKS[ci]
            nc.sync.dma_start(
                dst[:].rearrange("p (t c) -> p t c", t=8),
                xTv[:, 8 * half:8 * half + 8, start:start + cw])

        def x_load(ci):
            cw = CHUNKS[ci][1]
            xlo = xpool.tile([128, 8 * cw], F16, tag="xlo", name=f"xlo{ci}")
            x_dma(ci, 0, xlo)
            xhi = xpool.tile([128, 8 * cw], F16, tag="xhi", name=f"xhi{ci}")
            x_dma(ci, 1, xhi)
            xc_t[ci] = (xlo, xhi)

        x0lo = xpool.tile([128, HW], F16, tag="xlo", name="xlo0")
        x_dma(0, 0, x0lo)
        nc.sync.dma_start(split8(wk_hi[:]), wkv[:, 8:16, :])
        x0hi = xpool.tile([128, HW], F16, tag="xhi", name="xhi0")
        x_dma(0, 1, x0hi)
        xc_t[0] = (x0lo, x0hi)
        nc.sync.dma_start(split8(wq_lo[:]), wqv[:, 0:8, :])
        nc.sync.dma_start(split8(wq_hi[:]), wqv[:, 8:16, :])
        nc.sync.dma_start(
            wv_t[:].rearrange("p (t c) -> p t c", t=16), wvv)
        csk = cpool.tile([HD, S], F16, name="cosk")
        nc.sync.dma_start(csk[:], cosk)
        snk = cpool.tile([HD, S], F16, name="sink")
        nc.sync.dma_start(snk[:], sink)
        csq = cpool.tile([HD, S], F16, name="cosq")
        nc.sync.dma_start(csq[:], cosq)
        snq = cpool.tile([HD, S], F16, name="sinq")
        nc.sync.dma_start(snq[:], sinq)

        k_t = [kres.tile([HD, S], F16, tag="k", name=f"k{h}")
               for h in range(HPC)]
        v_t = [vres.tile([128, EL], F16, tag="v", name=f"v{st}")
               for st in range(S // 128)]
        z_part = [dram.tile([D, cw], F16, tag=f"zp{ci}", name=f"zp{ci}")
                  for ci, (_, cw) in enumerate(CHUNKS)]
        z_red = [dram.tile([EL, cw], F16, tag=f"zr{ci}", name=f"zr{ci}")
                 for ci, (_, cw) in enumerate(CHUNKS)]

        def rope_pre(ps, cw):
            """Phase A: drain the QKV PSUM chain to SBUF (frees the bank so
            the next PE chain never waits on downstream RoPE progress)."""
            pre = prepool.tile([128, cw], F16, tag="pre")
            nc.scalar.copy(pre[:], ps[:])
            return pre

        def rope_apply(pre, cw, cs, sn, out_tile, sl):
            """Phase B: out = pre*cos + rotate_half(pre)*sin (sign in sin)."""
            rot = rpool.tile([128, cw], F16, tag="rot")
            nc.sync.dma_start(rot[0:64, :], pre[64:128, :])
            nc.sync.dma_start(rot[64:128, :], pre[0:64, :])
            t1 = rpool.tile([128, cw], F16, tag="t1")
            nc.vector.tensor_mul(t1[:], pre[:], cs)
            t2 = rpool.tile([128, cw], F16, tag="t2")
            nc.vector.tensor_mul(t2[:], rot[:], sn)
            nc.vector.tensor_add(out_tile[:, sl], t1[:], t2[:])

        def proj_chunk(ci, y_sb):
            """Partial projection zp[ci] = Wp[g-rows]^T @ y, spill, and
            ReduceScatter. Emitted one chunk late (see module docstring)."""
            cw = CHUNKS[ci][1]
            for half in range(2):
                zb = zpool.tile([128, 8 * cw], F16, tag="zb")
                for dd in range(8):
                    d2 = 8 * half + dd
                    ps = ps_mm.tile([128, cw], F32)
                    for kk in range(KT):
                        nc.tensor.matmul(
                            ps[:],
                            wp_t[:, kk * D + d2 * 128:kk * D + (d2 + 1) * 128],
                            y_sb[kk][:], start=(kk == 0), stop=(kk == KT - 1))
                    nc.scalar.copy(zb[:, dd * cw:(dd + 1) * cw], ps[:])
                dst = z_part[ci][half * 8 * 128:(half + 1) * 8 * 128, :]
                nc.scalar.dma_start(
                    dst.rearrange("(t p) c -> p t c", p=128),
                    zb[:].rearrange("p (t c) -> p t c", t=8))
            nc.gpsimd.collective_compute(
                "ReduceScatter", mybir.AluOpType.add,
                replica_groups=GROUPS,
                ins=[z_part[ci].opt()], outs=[z_red[ci].opt()])

        pending_proj = None
        for ci, (start, cw) in enumerate(CHUNKS):
            tsl = slice(start, start + cw)
            if ci + 1 < len(CHUNKS):
                # prefetch next chunk's x ahead of the bulkier wp load so the
                # in-order DMA queue matches consumption order
                x_load(ci + 1)
            if ci == 0:
                nc.sync.dma_start(
                    wp_t[:].rearrange("p (k c) -> p k c", k=KT), wpv)
            xlo, xhi = xc_t[ci]

            # ---------------- K chunk (+RoPE) ----------------
            k_pre = []
            for h in range(HPC):
                ps = ps_mm.tile([HD, cw], F32)
                for dt in range(DT):
                    nc.tensor.matmul(
                        ps[:],
                        xw_slice(wk_lo, wk_hi, dt, CH, h * HD, (h + 1) * HD),
                        xw_slice(xlo, xhi, dt, cw, 0, cw),
                        start=(dt == 0), stop=(dt == DT - 1))
                k_pre.append(rope_pre(ps, cw))

            # ---------------- Q chunk (+RoPE) ----------------
            qc = []
            q_pre = []
            for h in range(HPC):
                ps = ps_mm.tile([HD, cw], F32)
                for dt in range(DT):
                    nc.tensor.matmul(
                        ps[:],
                        xw_slice(wq_lo, wq_hi, dt, CH, h * HD, (h + 1) * HD),
                        xw_slice(xlo, xhi, dt, cw, 0, cw),
                        start=(dt == 0), stop=(dt == DT - 1))
                q_pre.append(rope_pre(ps, cw))
                qc.append(qpool.tile([HD, cw], F16, tag="q", name=f"q{ci}_{h}"))
            for h in range(HPC):
                rope_apply(k_pre[h], cw, csk[:, tsl], snk[:, tsl], k_t[h], tsl)
                rope_apply(q_pre[h], cw, csq[:, tsl], snq[:, tsl], qc[h],
                           slice(None))

            # ---------------- V chunk (token-major) ----------------
            for st in range(cw // 128):
                ps = ps_mm.tile([128, EL], F32)
                for dt in range(DT):
                    nc.tensor.matmul(
                        ps[:],
                        xw_slice(xlo, xhi, dt, cw, st * 128, (st + 1) * 128),
                        wv_t[:, dt * EL:(dt + 1) * EL],
                        start=(dt == 0), stop=(dt == DT - 1))
                nc.scalar.copy(v_t[start // 128 + st][:], ps[:])

            # previous chunk's projection: fills the PE while this chunk's
            # RoPE completes, and hides the previous normalize latency
            if pending_proj is not None:
                proj_chunk(*pending_proj)

            # ---------------- causal attention for this chunk ----------------
            y_sb = []
            n_jt = (start + cw) // 128
            for h in range(HPC):
                o_ps = ps_o.tile([HD, cw], F32)
                r_ps = ps_r.tile([1, cw], F32)
                for jt in range(n_jt):
                    blk = jt * 128 - start
                    off = blk if blk > 0 else 0
                    s_ps = ps_sc.tile([128, cw], F32)
                    nc.tensor.matmul(
                        s_ps[:, off:], k_t[h][:, jt * 128:(jt + 1) * 128],
                        qc[h][:, off:], start=True, stop=True)
                    p = ppool.tile([128, cw], F16, tag="p")
                    nc.scalar.activation(p[:, off:], s_ps[:, off:], AF.Exp)
                    if blk >= 0:
                        nc.vector.tensor_mul(
                            p[:, off:off + 128], p[:, off:off + 128], tri_t[:])
                    nc.tensor.matmul(
                        o_ps[:, off:], v_t[jt][:, h * HD:(h + 1) * HD],
                        p[:, off:], start=(jt == 0), stop=(jt == n_jt - 1))
                    nc.tensor.matmul(
                        r_ps[:, off:], ones_t[:], p[:, off:],
                        start=(jt == 0), stop=(jt == n_jt - 1))
                rinv = spool.tile([1, cw], F16, tag="rinv")
                nc.vector.reciprocal(rinv[:], r_ps[:])
                bcast = spool.tile([128, cw], F16, tag="bcast")
                nc.gpsimd.partition_broadcast(bcast[:], rinv[:], channels=128)
                ot = opool.tile([HD, cw], F16, tag="o", name=f"o{ci}_{h}")
                nc.vector.tensor_copy(ot[:], o_ps[:])
                yt = ypool.tile([HD, cw], F16, tag="y", name=f"y{ci}_{h}")
                nc.vector.tensor_mul(yt[:], ot[:], bcast[:])
                y_sb.append(yt)
            pending_proj = (ci, y_sb)

        proj_chunk(*pending_proj)

        for ci, (start, cw) in enumerate(CHUNKS):
            nc.sync.dma_start(zc[:, start:start + cw], z_red[ci][:])
    nc.compile()
    return nc


def _tables():
    inv_freq = 1.0 / (ROPE_THETA ** (np.arange(0, HD, 2, dtype=np.float64) / HD))
    pos = np.arange(S, dtype=np.float64)
    f_half = np.outer(inv_freq, pos)                  # [64, S]
    freqs = np.concatenate([f_half, f_half], axis=0)  # [HD, S]
    # match reference numerics: cos/sin computed in float32 domain
    emb32 = freqs.astype(np.float32)
    cos_t = np.cos(emb32)
    sin_t = np.sin(emb32)
    scale = np.float32(HD ** -0.5)
    sgn = np.where(np.arange(HD) < HD // 2, -1.0, 1.0).astype(np.float32)[:, None]
    cosq = (cos_t * scale).astype(np.float16)
    sinq = (sin_t * sgn * scale).astype(np.float16)
    cosk = cos_t.astype(np.float16)
    sink = (sin_t * sgn).astype(np.float16)
    return cosq, sinq, cosk, sink


_NC_CACHE = {}


def _get_nc():
    if "nc" not in _NC_CACHE:
        _NC_CACHE["nc"] = _build()
    return _NC_CACHE["nc"]


def make_in_maps(x, W_attn, W_proj):
    x = np.asarray(x, dtype=np.float32)
    W_attn = np.asarray(W_attn, dtype=np.float32)
    W_proj = np.asarray(W_proj, dtype=np.float32)
    cosq, sinq, cosk, sink = _tables()
    tri = np.triu(np.ones((128, 128), np.float16))   # [jj, ii]: keep jj <= ii
    ones = np.ones((128, 1), np.float16)
    in_maps = []
    for c in range(N_CORES):
        b, g = divmod(c, HPC)
        in_maps.append({
            "xT": np.ascontiguousarray(x[b].T).astype(np.float16),
            "wq": W_attn[:, g * EL:(g + 1) * EL].astype(np.float16),
            "wk": W_attn[:, D + g * EL:D + (g + 1) * EL].astype(np.float16),
            "wv": W_attn[:, 2 * D + g * EL:2 * D + (g + 1) * EL].astype(np.float16),
            "wp": W_proj[g * EL:(g + 1) * EL, :].astype(np.float16),
            "cosq": cosq, "sinq": sinq, "cosk": cosk, "sink": sink,
            "tri": tri, "ones": ones,
        })
    return in_maps


def assemble(results):
    out = np.empty((B, S, D), dtype=np.float32)
    for c in range(N_CORES):
        b, g = divmod(c, HPC)
        zT = np.asarray(results[c]["zc"], dtype=np.float32)
        out[b, :, g * EL:(g + 1) * EL] = zT.T
    return out


def kernel(x, W_attn, W_proj):
    nc = _get_nc()
    in_maps = make_in_maps(x, W_attn, W_proj)
    res = bass_utils.run_bass_kernel_spmd(
        nc, in_maps, core_ids=list(range(N_CORES)), trace=False)
    return assemble(res.results)


if __name__ == "__main__":
    rng = np.random.default_rng(0)
    x = rng.standard_normal((B, S, D)).astype(np.float32)
    W_attn = (rng.standard_normal((D, 3 * D)) * D ** -0.5).astype(np.float32)
    W_proj = (rng.standard_normal((D, D)) * D ** -0.5).astype(np.float32)
    out = kernel(x, W_attn, W_proj)
    print("out", out.shape, out.dtype, np.abs(out).mean())


# revision 28
# speedup vs baseline: 2.3562x; 1.0038x over previous
"""Causal self-attention with RoPE on 8 TRN2 NeuronCores.

Sharding: core c -> (batch b = c//4, head-group g = c%4; 4 heads of 128 each).
Tensor-parallel over heads x data-parallel over batch.

Single fused pipeline per core, one pass over x, token chunks of
[512, 512, 512, 256, 256]. Per chunk: QKV matmuls (+RoPE on Q,K) -> causal
attention over kv-blocks <= chunk end -> project own heads through own W_proj
row-block into a full-D partial z -> per-chunk ReduceScatter(add) within the
batch group produces this core's final 512-dim slice of z^T.

The projection for chunk i is EMITTED inside chunk i+1 (after its QKV
matmuls): the softmax-normalize latency chain (rowsum -> reciprocal ->
broadcast -> scale) then hides under 40us of next-chunk PE work instead of
stalling the in-order PE at every chunk boundary. The two final 256-token
chunks shrink the only exposed collective (last ReduceScatter: 21.5us vs
28us) and the tail projection.

All matmul operands are fp16 (1 PE cycle/row, same as bf16, 10-bit
mantissa). PSUM accumulation and softmax statistics stay fp32.

DMA queues: sync = input prefetch in consumption order + RoPE rotate-half
swaps; Act = z-partial spills (their producer is the Act PSUM-drain copy);
Pool = collectives + 1/rowsum partition_broadcast. Big input transfers are
batched (2 DMAs per x chunk / weight matrix) but halved so the first
accumulation chain can start after ~6us.

Layouts (no on-chip transposes needed):
  xT   [D, S]   = x[b].T                      (host-transposed)
  Q^T,K^T [128, S] per head  (from matmul: lhsT=W-block, rhs=xT)
  V    [S, 512] token-major  (from matmul: lhsT=xT-tile, rhs=Wv)
  S^T  [j, i] scores blocks -> softmax sums via ones-matmul on PE
  O^T  [c, i] accumulated in PSUM, normalized by 1/rowsum afterwards
  zp   [D, cw] partial projection, ReduceScatter -> z_red [512, cw] -> zT
"""
from contextlib import ExitStack

import numpy as np

import concourse.bass as bass
import concourse.tile as tile
import concourse.mybir as mybir
from concourse import bacc, bass_utils

B = 2
S = 2048
D = 2048
NH, HD = 16, 128
HPC = 4                 # heads per core
EL = HPC * HD           # 512: local e-width per core
CH = 512                # max token-chunk width
CHUNKS = [(0, 512), (512, 512), (1024, 512), (1536, 256), (1792, 256)]
DT = D // 128           # 16 d-tiles
KT = EL // 128          # 4 k-tiles of the local proj contraction
ROPE_THETA = 10000.0
N_CORES = 8
GROUPS = [[0, 1, 2, 3], [4, 5, 6, 7]]

F32 = mybir.dt.float32
F16 = mybir.dt.float16
AF = mybir.ActivationFunctionType


def _build():
    nc = bacc.Bacc("TRN2", target_bir_lowering=False, debug=False,
                   enable_asserts=True, num_devices=N_CORES)
    xT = nc.dram_tensor("xT", [D, S], F16, kind="ExternalInput").ap()
    wq = nc.dram_tensor("wq", [D, EL], F16, kind="ExternalInput").ap()
    wk = nc.dram_tensor("wk", [D, EL], F16, kind="ExternalInput").ap()
    wv = nc.dram_tensor("wv", [D, EL], F16, kind="ExternalInput").ap()
    wp = nc.dram_tensor("wp", [EL, D], F16, kind="ExternalInput").ap()
    cosq = nc.dram_tensor("cosq", [HD, S], F16, kind="ExternalInput").ap()
    sinq = nc.dram_tensor("sinq", [HD, S], F16, kind="ExternalInput").ap()
    cosk = nc.dram_tensor("cosk", [HD, S], F16, kind="ExternalInput").ap()
    sink = nc.dram_tensor("sink", [HD, S], F16, kind="ExternalInput").ap()
    tri = nc.dram_tensor("tri", [128, 128], F16, kind="ExternalInput").ap()
    ones = nc.dram_tensor("ones", [128, 1], F16, kind="ExternalInput").ap()
    zc = nc.dram_tensor("zc", [EL, S], F16, kind="ExternalOutput").ap()

    # [p, t, e] views: 128 partitions, d-tiles stacked along a middle dim
    wqv = wq.rearrange("(t p) e -> p t e", p=128)
    wkv = wk.rearrange("(t p) e -> p t e", p=128)
    wvv = wv.rearrange("(t p) e -> p t e", p=128)
    wpv = wp.rearrange("(k p) d -> p k d", p=128)
    xTv = xT.rearrange("(t p) s -> p t s", p=128)

    HW = 8 * CH          # half-width of a batched x / qkv-weight transfer

    with tile.TileContext(nc) as tc, \
         nc.allow_low_precision(reason="fp16 attention"), ExitStack() as ctx:
        cpool = ctx.enter_context(tc.tile_pool(name="const", bufs=1))
        wpool = ctx.enter_context(tc.tile_pool(name="wts", bufs=1))
        xpool = ctx.enter_context(tc.tile_pool(name="xc", bufs=2))
        kres = ctx.enter_context(tc.tile_pool(name="kres", bufs=4))
        vres = ctx.enter_context(tc.tile_pool(name="vres", bufs=16))
        qpool = ctx.enter_context(tc.tile_pool(name="qp", bufs=5))
        prepool = ctx.enter_context(tc.tile_pool(name="pre", bufs=8))
        rpool = ctx.enter_context(tc.tile_pool(name="rope", bufs=2))
        ppool = ctx.enter_context(tc.tile_pool(name="pp", bufs=4))
        opool = ctx.enter_context(tc.tile_pool(name="op", bufs=4))
        spool = ctx.enter_context(tc.tile_pool(name="sm", bufs=3))
        ypool = ctx.enter_context(tc.tile_pool(name="yp", bufs=9))
        zpool = ctx.enter_context(tc.tile_pool(name="zp", bufs=2))
        dram = ctx.enter_context(tc.tile_pool(name="dram", bufs=1, space="DRAM"))
        ps_mm = ctx.enter_context(tc.tile_pool(name="ps_mm", bufs=2, space="PSUM"))
        ps_sc = ctx.enter_context(tc.tile_pool(name="ps_sc", bufs=3, space="PSUM"))
        ps_o = ctx.enter_context(tc.tile_pool(name="ps_o", bufs=2, space="PSUM"))
        ps_r = ctx.enter_context(tc.tile_pool(name="ps_r", bufs=1, space="PSUM"))

        # ---- prefetch stream (sync queue order == arrival priority) ----
        tri_t = cpool.tile([128, 128], F16)
        nc.sync.dma_start(tri_t[:], tri)
        ones_t = cpool.tile([128, 1], F16)
        nc.sync.dma_start(ones_t[:], ones)

        wk_lo = wpool.tile([128, HW], F16, name="wk_lo")
        wk_hi = wpool.tile([128, HW], F16, name="wk_hi")
        wq_lo = wpool.tile([128, HW], F16, name="wq_lo")
        wq_hi = wpool.tile([128, HW], F16, name="wq_hi")
        wv_t = wpool.tile([128, 2 * HW], F16, name="wv")
        wp_t = wpool.tile([128, KT * D], F16, name="wp")

        def xw_slice(lo, hi, dt, cw, c0, c1):
            t = lo if dt < 8 else hi
            base = (dt % 8) * cw
            return t[:, base + c0:base + c1]

        def split8(ap):
            return ap.rearrange("p (t c) -> p t c", t=8)

        nc.sync.dma_start(split8(wk_lo[:]), wkv[:, 0:8, :])
        xc_t = {}

        def x_dma(ci, half, dst):
            # one DMA covering 8 d-tiles x cw tokens of chunk ci
            start, cw = CHUNKS[ci]
            nc.sync.dma_start(
                dst[:].rearrange("p (t c) -> p t c", t=8),
                xTv[:, 8 * half:8 * half + 8, start:start + cw])

        def x_load(ci):
            cw = CHUNKS[ci][1]
            xlo = xpool.tile([128, 8 * cw], F16, tag="xlo", name=f"xlo{ci}")
            x_dma(ci, 0, xlo)
            xhi = xpool.tile([128, 8 * cw], F16, tag="xhi", name=f"xhi{ci}")
            x_dma(ci, 1, xhi)
            xc_t[ci] = (xlo, xhi)

        x0lo = xpool.tile([128, HW], F16, tag="xlo", name="xlo0")
        x_dma(0, 0, x0lo)
        nc.sync.dma_start(split8(wk_hi[:]), wkv[:, 8:16, :])
        x0hi = xpool.tile([128, HW], F16, tag="xhi", name="xhi0")
        x_dma(0, 1, x0hi)
        xc_t[0] = (x0lo, x0hi)
        nc.sync.dma_start(split8(wq_lo[:]), wqv[:, 0:8, :])
        nc.sync.dma_start(split8(wq_hi[:]), wqv[:, 8:16, :])
        nc.sync.dma_start(
            wv_t[:].rearrange("p (t c) -> p t c", t=16), wvv)
        csk = cpool.tile([HD, S], F16, name="cosk")
        nc.sync.dma_start(csk[:], cosk)
        snk = cpool.tile([HD, S], F16, name="sink")
        nc.sync.dma_start(snk[:], sink)
        csq = cpool.tile([HD, S], F16, name="cosq")
        nc.sync.dma_start(csq[:], cosq)
        snq = cpool.tile([HD, S], F16, name="sinq")
        nc.sync.dma_start(snq[:], sinq)

        k_t = [kres.tile([HD, S], F16, tag="k", name=f"k{h}")
               for h in range(HPC)]
        v_t = [vres.tile([128, EL], F16, tag="v", name=f"v{st}")
               for st in range(S // 128)]
        z_part = [dram.tile([D, cw], F16, tag=f"zp{ci}", name=f"zp{ci}")
                  for ci, (_, cw) in enumerate(CHUNKS)]
        z_red = [dram.tile([EL, cw], F16, tag=f"zr{ci}", name=f"zr{ci}")
                 for ci, (_, cw) in enumerate(CHUNKS)]

        def rope_pre(ps, cw):
            """Phase A: drain the QKV PSUM chain to SBUF (frees the bank so
            the next PE chain never waits on downstream RoPE progress)."""
            pre = prepool.tile([128, cw], F16, tag="pre")
            nc.scalar.copy(pre[:], ps[:])
            return pre

        def rope_apply(pre, cw, cs, sn, out_tile, sl):
            """Phase B: out = pre*cos + rotate_half(pre)*sin (sign in sin)."""
            rot = rpool.tile([128, cw], F16, tag="rot")
            nc.sync.dma_start(rot[0:64, :], pre[64:128, :])
            nc.sync.dma_start(rot[64:128, :], pre[0:64, :])
            t1 = rpool.tile([128, cw], F16, tag="t1")
            nc.vector.tensor_mul(t1[:], pre[:], cs)
            t2 = rpool.tile([128, cw], F16, tag="t2")
            nc.vector.tensor_mul(t2[:], rot[:], sn)
            nc.vector.tensor_add(out_tile[:, sl], t1[:], t2[:])

        def proj_chunk(ci, y_sb):
            """Partial projection zp[ci] = Wp[g-rows]^T @ y, spill, and
            ReduceScatter. Emitted one chunk late (see module docstring)."""
            cw = CHUNKS[ci][1]
            for half in range(2):
                zb = zpool.tile([128, 8 * cw], F16, tag="zb")
                for dd in range(8):
                    d2 = 8 * half + dd
                    ps = ps_mm.tile([128, cw], F32)
                    for kk in range(KT):
                        nc.tensor.matmul(
                            ps[:],
                            wp_t[:, kk * D + d2 * 128:kk * D + (d2 + 1) * 128],
                            y_sb[kk][:], start=(kk == 0), stop=(kk == KT - 1))
                    nc.scalar.copy(zb[:, dd * cw:(dd + 1) * cw], ps[:])
                dst = z_part[ci][half * 8 * 128:(half + 1) * 8 * 128, :]
                nc.scalar.dma_start(
                    dst.rearrange("(t p) c -> p t c", p=128),
                    zb[:].rearrange("p (t c) -> p t c", t=8))
            nc.gpsimd.collective_compute(
                "ReduceScatter", mybir.AluOpType.add,
                replica_groups=GROUPS,
                ins=[z_part[ci].opt()], outs=[z_red[ci].opt()])

        pending_proj = None
        for ci, (start, cw) in enumerate(CHUNKS):
            tsl = slice(start, start + cw)
            if ci + 1 < len(CHUNKS):
                # prefetch next chunk's x ahead of the bulkier wp load so the
                # in-order DMA queue matches consumption order
                x_load(ci + 1)
            if ci == 0:
                nc.sync.dma_start(
                    wp_t[:].rearrange("p (k c) -> p k c", k=KT), wpv)
            xlo, xhi = xc_t[ci]

            # ---------------- K chunk (+RoPE) ----------------
            k_pre = []
            for h in range(HPC):
                ps = ps_mm.tile([HD, cw], F32)
                for dt in range(DT):
                    nc.tensor.matmul(
                        ps[:],
                        xw_slice(wk_lo, wk_hi, dt, CH, h * HD, (h + 1) * HD),
                        xw_slice(xlo, xhi, dt, cw, 0, cw),
                        start=(dt == 0), stop=(dt == DT - 1))
                k_pre.append(rope_pre(ps, cw))

            # ---------------- Q chunk (+RoPE) ----------------
            qc = []
            q_pre = []
            for h in range(HPC):
                ps = ps_mm.tile([HD, cw], F32)
                for dt in range(DT):
                    nc.tensor.matmul(
                        ps[:],
                        xw_slice(wq_lo, wq_hi, dt, CH, h * HD, (h + 1) * HD),
                        xw_slice(xlo, xhi, dt, cw, 0, cw),
                        start=(dt == 0), stop=(dt == DT - 1))
                q_pre.append(rope_pre(ps, cw))
                qc.append(qpool.tile([HD, cw], F16, tag="q", name=f"q{ci}_{h}"))
            for h in range(HPC):
                rope_apply(k_pre[h], cw, csk[:, tsl], snk[:, tsl], k_t[h], tsl)
                rope_apply(q_pre[h], cw, csq[:, tsl], snq[:, tsl], qc[h],
                           slice(None))

            # ---------------- V chunk (token-major) ----------------
            for st in range(cw // 128):
                ps = ps_mm.tile([128, EL], F32)
                for dt in range(DT):
                    nc.tensor.matmul(
                        ps[:],
                        xw_slice(xlo, xhi, dt, cw, st * 128, (st + 1) * 128),
                        wv_t[:, dt * EL:(dt + 1) * EL],
                        start=(dt == 0), stop=(dt == DT - 1))
                nc.scalar.copy(v_t[start // 128 + st][:], ps[:])

            # previous chunk's projection: fills the PE while this chunk's
            # RoPE completes, and hides the previous normalize latency
            if pending_proj is not None:
                proj_chunk(*pending_proj)

            # ---------------- causal attention for this chunk ----------------
            y_sb = []
            n_jt = (start + cw) // 128
            for h in range(HPC):
                o_ps = ps_o.tile([HD, cw], F32)
                r_ps = ps_r.tile([1, cw], F32)
                for jt in range(n_jt):
                    blk = jt * 128 - start
                    off = blk if blk > 0 else 0
                    s_ps = ps_sc.tile([128, cw], F32)
                    nc.tensor.matmul(
                        s_ps[:, off:], k_t[h][:, jt * 128:(jt + 1) * 128],
                        qc[h][:, off:], start=True, stop=True)
                    p = ppool.tile([128, cw], F16, tag="p")
                    nc.scalar.activation(p[:, off:], s_ps[:, off:], AF.Exp)
                    if blk >= 0:
                        nc.vector.tensor_mul(
                            p[:, off:off + 128], p[:, off:off + 128], tri_t[:])
                    nc.tensor.matmul(
                        o_ps[:, off:], v_t[jt][:, h * HD:(h + 1) * HD],
                        p[:, off:], start=(jt == 0), stop=(jt == n_jt - 1))
                    nc.tensor.matmul(
                        r_ps[:, off:], ones_t[:], p[:, off:],
                        start=(jt == 0), stop=(jt == n_jt - 1))
                rinv = spool.tile([1, cw], F16, tag="rinv")
                nc.vector.reciprocal(rinv[:], r_ps[:])
                bcast = spool.tile([128, cw], F16, tag="bcast")
                nc.gpsimd.partition_broadcast(bcast[:], rinv[:], channels=128)
                ot = opool.tile([HD, cw], F16, tag="o", name=f"o{ci}_{h}")
                nc.vector.tensor_copy(ot[:], o_ps[:])
                yt = ypool.tile([HD, cw], F16, tag="y", name=f"y{ci}_{h}")
                nc.vector.tensor_mul(yt[:], ot[:], bcast[:])
                y_sb.append(yt)
            pending_proj = (ci, y_sb)

        proj_chunk(*pending_proj)

        for ci, (start, cw) in enumerate(CHUNKS):
            nc.sync.dma_start(zc[:, start:start + cw], z_red[ci][:])
    nc.compile()
    return nc


def _tables():
    inv_freq = 1.0 / (ROPE_THETA ** (np.arange(0, HD, 2, dtype=np.float64) / HD))
    pos = np.arange(S, dtype=np.float64)
    f_half = np.outer(inv_freq, pos)                  # [64, S]
    freqs = np.concatenate([f_half, f_half], axis=0)  # [HD, S]
    # match reference numerics: cos/sin computed in float32 domain
    emb32 = freqs.astype(np.float32)
    cos_t = np.cos(emb32)
    sin_t = np.sin(emb32)
    scale = np.float32(HD ** -0.5)
    sgn = np.where(np.arange(HD) < HD // 2, -1.0, 1.0).astype(np.float32)[:, None]
    cosq = (cos_t * scale).astype(np.float16)
    sinq = (sin_t * sgn * scale).astype(np.float16)
    cosk = cos_t.astype(np.float16)
    sink = (sin_t * sgn).astype(np.float16)
    return cosq, sinq, cosk, sink


_NC_CACHE = {}


def _get_nc():
    if "nc" not in _NC_CACHE:
        _NC_CACHE["nc"] = _build()
    return _NC_CACHE["nc"]


def make_in_maps(x, W_attn, W_proj):
    x = np.asarray(x, dtype=np.float32)
    W_attn = np.asarray(W_attn, dtype=np.float32)
    W_proj = np.asarray(W_proj, dtype=np.float32)
    cosq, sinq, cosk, sink = _tables()
    tri = np.triu(np.ones((128, 128), np.float16))   # [jj, ii]: keep jj <= ii
    ones = np.ones((128, 1), np.float16)
    in_maps = []
    for c in range(N_CORES):
        b, g = divmod(c, HPC)
        in_maps.append({
            "xT": np.ascontiguousarray(x[b].T).astype(np.float16),
            "wq": W_attn[:, g * EL:(g + 1) * EL].astype(np.float16),
            "wk": W_attn[:, D + g * EL:D + (g + 1) * EL].astype(np.float16),
            "wv": W_attn[:, 2 * D + g * EL:2 * D + (g + 1) * EL].astype(np.float16),
            "wp": W_proj[g * EL:(g + 1) * EL, :].astype(np.float16),
            "cosq": cosq, "sinq": sinq, "cosk": cosk, "sink": sink,
            "tri": tri, "ones": ones,
        })
    return in_maps


def assemble(results):
    out = np.empty((B, S, D), dtype=np.float32)
    for c in range(N_CORES):
        b, g = divmod(c, HPC)
        zT = np.asarray(results[c]["zc"], dtype=np.float32)
        out[b, :, g * EL:(g + 1) * EL] = zT.T
    return out


def kernel(x, W_attn, W_proj):
    nc = _get_nc()
    in_maps = make_in_maps(x, W_attn, W_proj)
    res = bass_utils.run_bass_kernel_spmd(
        nc, in_maps, core_ids=list(range(N_CORES)), trace=False)
    return assemble(res.results)


if __name__ == "__main__":
    rng = np.random.default_rng(0)
    x = rng.standard_normal((B, S, D)).astype(np.float32)
    W_attn = (rng.standard_normal((D, 3 * D)) * D ** -0.5).astype(np.float32)
    W_proj = (rng.standard_normal((D, D)) * D ** -0.5).astype(np.float32)
    out = kernel(x, W_attn, W_proj)
    print("out", out.shape, out.dtype, np.abs(out).mean())


# revision 35
# speedup vs baseline: 2.3801x; 1.0101x over previous
"""Causal self-attention with RoPE on 8 TRN2 NeuronCores.

Sharding: core c -> (batch b = c//4, head-group g = c%4; 4 heads of 128 each).
Tensor-parallel over heads x data-parallel over batch.

Single fused pipeline per core, one pass over x, token chunks of
[512, 512, 512, 256, 256]. Per chunk: QKV matmuls (+RoPE on Q,K) -> causal
attention over kv-blocks <= chunk end -> project own heads through own W_proj
row-block into a full-D partial z -> per-chunk ReduceScatter(add) within the
batch group produces this core's final 512-dim slice of z^T.

The projection for chunk i is EMITTED inside chunk i+1 (after its QKV
matmuls): the softmax-normalize latency chain (rowsum -> reciprocal ->
broadcast -> scale) then hides under 40us of next-chunk PE work instead of
stalling the in-order PE at every chunk boundary. The two final 256-token
chunks shrink the only exposed collective (last ReduceScatter: 21.5us vs
28us) and the tail projection.

All matmul operands are fp16 (1 PE cycle/row, same as bf16, 10-bit
mantissa). PSUM accumulation and softmax statistics stay fp32.

DMA queues: sync = input prefetch in consumption order + RoPE rotate-half
swaps; Act = z-partial spills (their producer is the Act PSUM-drain copy);
Pool = collectives + 1/rowsum partition_broadcast. Big input transfers are
batched (2 DMAs per x chunk / weight matrix) but halved so the first
accumulation chain can start after ~6us.

Layouts (no on-chip transposes needed):
  xT   [D, S]   = x[b].T                      (host-transposed)
  Q^T,K^T [128, S] per head  (from matmul: lhsT=W-block, rhs=xT)
  V    [S, 512] token-major  (from matmul: lhsT=xT-tile, rhs=Wv)
  S^T  [j, i] scores blocks -> softmax sums via ones-matmul on PE
  O^T  [c, i] accumulated in PSUM, normalized by 1/rowsum afterwards
  zp   [D, cw] partial projection, ReduceScatter -> z_red [512, cw] -> zT
"""
from contextlib import ExitStack

import numpy as np

import concourse.bass as bass
import concourse.tile as tile
import concourse.mybir as mybir
from concourse import bacc, bass_utils

B = 2
S = 2048
D = 2048
NH, HD = 16, 128
HPC = 4                 # heads per core
EL = HPC * HD           # 512: local e-width per core
CH = 512                # max token-chunk width
CHUNKS = [(0, 512), (512, 512), (1024, 512), (1536, 384), (1920, 128)]
DT = D // 128           # 16 d-tiles
KT = EL // 128          # 4 k-tiles of the local proj contraction
ROPE_THETA = 10000.0
N_CORES = 8
GROUPS = [[0, 1, 2, 3], [4, 5, 6, 7]]

F32 = mybir.dt.float32
F16 = mybir.dt.float16
AF = mybir.ActivationFunctionType


def _build():
    nc = bacc.Bacc("TRN2", target_bir_lowering=False, debug=False,
                   enable_asserts=True, num_devices=N_CORES)
    xT = nc.dram_tensor("xT", [D, S], F16, kind="ExternalInput").ap()
    # wq/wk arrive host pre-tiled: [128 partitions, (d-tile, e)] so any
    # column range is a contiguous >=4KB run per partition
    wq = nc.dram_tensor("wq", [128, DT * EL], F16, kind="ExternalInput").ap()
    wk = nc.dram_tensor("wk", [128, DT * EL], F16, kind="ExternalInput").ap()
    wv = nc.dram_tensor("wv", [D, EL], F16, kind="ExternalInput").ap()
    wp = nc.dram_tensor("wp", [EL, D], F16, kind="ExternalInput").ap()
    cosq = nc.dram_tensor("cosq", [HD, S], F16, kind="ExternalInput").ap()
    sinq = nc.dram_tensor("sinq", [HD, S], F16, kind="ExternalInput").ap()
    cosk = nc.dram_tensor("cosk", [HD, S], F16, kind="ExternalInput").ap()
    sink = nc.dram_tensor("sink", [HD, S], F16, kind="ExternalInput").ap()
    tri = nc.dram_tensor("tri", [128, 128], F16, kind="ExternalInput").ap()
    ones = nc.dram_tensor("ones", [128, 1], F16, kind="ExternalInput").ap()
    zc = nc.dram_tensor("zc", [EL, S], F16, kind="ExternalOutput").ap()

    # [p, t, e] views: 128 partitions, d-tiles stacked along a middle dim
    wvv = wv.rearrange("(t p) e -> p t e", p=128)
    wpv = wp.rearrange("(k p) d -> p k d", p=128)
    xTv = xT.rearrange("(t p) s -> p t s", p=128)

    HW = 8 * CH          # half-width of a batched x / qkv-weight transfer

    with tile.TileContext(nc) as tc, \
         nc.allow_low_precision(reason="fp16 attention"), ExitStack() as ctx:
        cpool = ctx.enter_context(tc.tile_pool(name="const", bufs=1))
        wpool = ctx.enter_context(tc.tile_pool(name="wts", bufs=1))
        xpool = ctx.enter_context(tc.tile_pool(name="xc", bufs=2))
        kres = ctx.enter_context(tc.tile_pool(name="kres", bufs=4))
        vres = ctx.enter_context(tc.tile_pool(name="vres", bufs=16))
        qpool = ctx.enter_context(tc.tile_pool(name="qp", bufs=5))
        prepool = ctx.enter_context(tc.tile_pool(name="pre", bufs=8))
        rpool = ctx.enter_context(tc.tile_pool(name="rope", bufs=2))
        ppool = ctx.enter_context(tc.tile_pool(name="pp", bufs=4))
        opool = ctx.enter_context(tc.tile_pool(name="op", bufs=4))
        spool = ctx.enter_context(tc.tile_pool(name="sm", bufs=3))
        ypool = ctx.enter_context(tc.tile_pool(name="yp", bufs=9))
        zpool = ctx.enter_context(tc.tile_pool(name="zp", bufs=2))
        dram = ctx.enter_context(tc.tile_pool(name="dram", bufs=1, space="DRAM"))
        ps_mm = ctx.enter_context(tc.tile_pool(name="ps_mm", bufs=2, space="PSUM"))
        ps_sc = ctx.enter_context(tc.tile_pool(name="ps_sc", bufs=3, space="PSUM"))
        ps_o = ctx.enter_context(tc.tile_pool(name="ps_o", bufs=2, space="PSUM"))
        ps_r = ctx.enter_context(tc.tile_pool(name="ps_r", bufs=1, space="PSUM"))

        # ---- prefetch stream (sync queue order == arrival priority) ----
        tri_t = cpool.tile([128, 128], F16)
        nc.sync.dma_start(tri_t[:], tri)
        ones_t = cpool.tile([128, 1], F16)
        nc.sync.dma_start(ones_t[:], ones)

        QW = 4 * EL          # quarter-width of a wk/wq transfer (4 d-tiles)
        wk_p = [wpool.tile([128, QW], F16, name=f"wk_p{i}") for i in range(4)]
        wq_p = [wpool.tile([128, QW], F16, name=f"wq_p{i}") for i in range(4)]
        wv_t = wpool.tile([128, 2 * HW], F16, name="wv")
        wp_t = wpool.tile([128, KT * D], F16, name="wp")

        def xw_slice(parts, dt, cw, c0, c1):
            base = (dt % 4) * cw
            return parts[dt // 4][:, base + c0:base + c1]

        xc_t = {}

        def x_load(ci, interleave=None):
            # 4 DMAs per chunk, each covering 4 d-tiles x cw tokens
            start, cw = CHUNKS[ci]
            parts = []
            for i in range(4):
                if interleave is not None:
                    interleave(i)
                xp = xpool.tile([128, 4 * cw], F16, tag=f"x{i}",
                                name=f"x{ci}_{i}")
                nc.sync.dma_start(
                    xp[:].rearrange("p (t c) -> p t c", t=4),
                    xTv[:, 4 * i:4 * i + 4, start:start + cw])
                parts.append(xp)
            xc_t[ci] = parts

        # interleave wk quarters with x0 quarters: the first K chain starts
        # after ~2 small transfers instead of the full 4.2MB
        x_load(0, interleave=lambda i: nc.sync.dma_start(
            wk_p[i][:], wk[:, i * QW:(i + 1) * QW]))
        for i in range(4):
            nc.sync.dma_start(wq_p[i][:], wq[:, i * QW:(i + 1) * QW])
        nc.sync.dma_start(
            wv_t[:].rearrange("p (t c) -> p t c", t=16), wvv)
        csk = cpool.tile([HD, S], F16, name="cosk")
        nc.sync.dma_start(csk[:], cosk)
        snk = cpool.tile([HD, S], F16, name="sink")
        nc.sync.dma_start(snk[:], sink)
        csq = cpool.tile([HD, S], F16, name="cosq")
        nc.sync.dma_start(csq[:], cosq)
        snq = cpool.tile([HD, S], F16, name="sinq")
        nc.sync.dma_start(snq[:], sinq)

        k_t = [kres.tile([HD, S], F16, tag="k", name=f"k{h}")
               for h in range(HPC)]
        v_t = [vres.tile([128, EL], F16, tag="v", name=f"v{st}")
               for st in range(S // 128)]
        z_part = [dram.tile([D, cw], F16, tag=f"zp{ci}", name=f"zp{ci}")
                  for ci, (_, cw) in enumerate(CHUNKS)]
        z_red = [dram.tile([EL, cw], F16, tag=f"zr{ci}", name=f"zr{ci}")
                 for ci, (_, cw) in enumerate(CHUNKS)]

        def rope_pre(ps, cw):
            """Phase A: drain the QKV PSUM chain to SBUF (frees the bank so
            the next PE chain never waits on downstream RoPE progress)."""
            pre = prepool.tile([128, cw], F16, tag="pre")
            nc.scalar.copy(pre[:], ps[:])
            return pre

        def rope_apply(pre, cw, cs, sn, out_tile, sl):
            """Phase B: out = pre*cos + rotate_half(pre)*sin (sign in sin)."""
            rot = rpool.tile([128, cw], F16, tag="rot")
            nc.sync.dma_start(rot[0:64, :], pre[64:128, :])
            nc.sync.dma_start(rot[64:128, :], pre[0:64, :])
            t1 = rpool.tile([128, cw], F16, tag="t1")
            nc.vector.tensor_mul(t1[:], pre[:], cs)
            t2 = rpool.tile([128, cw], F16, tag="t2")
            nc.vector.tensor_mul(t2[:], rot[:], sn)
            nc.vector.tensor_add(out_tile[:, sl], t1[:], t2[:])

        def proj_chunk(ci, y_sb):
            """Partial projection zp[ci] = Wp[g-rows]^T @ y, spill, and
            ReduceScatter. Emitted one chunk late (see module docstring)."""
            cw = CHUNKS[ci][1]
            for half in range(2):
                zb = zpool.tile([128, 8 * cw], F16, tag="zb")
                for dd in range(8):
                    d2 = 8 * half + dd
                    ps = ps_mm.tile([128, cw], F32)
                    for kk in range(KT):
                        nc.tensor.matmul(
                            ps[:],
                            wp_t[:, kk * D + d2 * 128:kk * D + (d2 + 1) * 128],
                            y_sb[kk][:], start=(kk == 0), stop=(kk == KT - 1))
                    nc.scalar.copy(zb[:, dd * cw:(dd + 1) * cw], ps[:])
                dst = z_part[ci][half * 8 * 128:(half + 1) * 8 * 128, :]
                nc.scalar.dma_start(
                    dst.rearrange("(t p) c -> p t c", p=128),
                    zb[:].rearrange("p (t c) -> p t c", t=8))
            nc.gpsimd.collective_compute(
                "ReduceScatter", mybir.AluOpType.add,
                replica_groups=GROUPS,
                ins=[z_part[ci].opt()], outs=[z_red[ci].opt()])

        pending_proj = None
        for ci, (start, cw) in enumerate(CHUNKS):
            tsl = slice(start, start + cw)
            if ci + 1 < len(CHUNKS):
                # prefetch next chunk's x ahead of the bulkier wp load so the
                # in-order DMA queue matches consumption order
                x_load(ci + 1)
            if ci == 0:
                nc.sync.dma_start(
                    wp_t[:].rearrange("p (k c) -> p k c", k=KT), wpv)
            xparts = xc_t[ci]

            # ---------------- K chunk (+RoPE) ----------------
            k_pre = []
            for h in range(HPC):
                ps = ps_mm.tile([HD, cw], F32)
                for dt in range(DT):
                    nc.tensor.matmul(
                        ps[:],
                        xw_slice(wk_p, dt, EL, h * HD, (h + 1) * HD),
                        xw_slice(xparts, dt, cw, 0, cw),
                        start=(dt == 0), stop=(dt == DT - 1))
                k_pre.append(rope_pre(ps, cw))

            # ---------------- Q chunk (+RoPE) ----------------
            qc = []
            q_pre = []
            for h in range(HPC):
                ps = ps_mm.tile([HD, cw], F32)
                for dt in range(DT):
                    nc.tensor.matmul(
                        ps[:],
                        xw_slice(wq_p, dt, EL, h * HD, (h + 1) * HD),
                        xw_slice(xparts, dt, cw, 0, cw),
                        start=(dt == 0), stop=(dt == DT - 1))
                q_pre.append(rope_pre(ps, cw))
                qc.append(qpool.tile([HD, cw], F16, tag="q", name=f"q{ci}_{h}"))
            for h in range(HPC):
                rope_apply(k_pre[h], cw, csk[:, tsl], snk[:, tsl], k_t[h], tsl)
                rope_apply(q_pre[h], cw, csq[:, tsl], snq[:, tsl], qc[h],
                           slice(None))

            # ---------------- V chunk (token-major) ----------------
            for st in range(cw // 128):
                ps = ps_mm.tile([128, EL], F32)
                for dt in range(DT):
                    nc.tensor.matmul(
                        ps[:],
                        xw_slice(xparts, dt, cw, st * 128, (st + 1) * 128),
                        wv_t[:, dt * EL:(dt + 1) * EL],
                        start=(dt == 0), stop=(dt == DT - 1))
                nc.scalar.copy(v_t[start // 128 + st][:], ps[:])

            # previous chunk's projection: fills the PE while this chunk's
            # RoPE completes, and hides the previous normalize latency
            if pending_proj is not None:
                proj_chunk(*pending_proj)

            # ---------------- causal attention for this chunk ----------------
            y_sb = []
            n_jt = (start + cw) // 128
            for h in range(HPC):
                o_ps = ps_o.tile([HD, cw], F32)
                r_ps = ps_r.tile([1, cw], F32)
                for jt in range(n_jt):
                    blk = jt * 128 - start
                    off = blk if blk > 0 else 0
                    s_ps = ps_sc.tile([128, cw], F32)
                    nc.tensor.matmul(
                        s_ps[:, off:], k_t[h][:, jt * 128:(jt + 1) * 128],
                        qc[h][:, off:], start=True, stop=True)
                    p = ppool.tile([128, cw], F16, tag="p")
                    nc.scalar.activation(p[:, off:], s_ps[:, off:], AF.Exp)
                    if blk >= 0:
                        nc.vector.tensor_mul(
                            p[:, off:off + 128], p[:, off:off + 128], tri_t[:])
                    nc.tensor.matmul(
                        o_ps[:, off:], v_t[jt][:, h * HD:(h + 1) * HD],
                        p[:, off:], start=(jt == 0), stop=(jt == n_jt - 1))
                    nc.tensor.matmul(
                        r_ps[:, off:], ones_t[:], p[:, off:],
                        start=(jt == 0), stop=(jt == n_jt - 1))
                rinv = spool.tile([1, cw], F16, tag="rinv")
                nc.vector.reciprocal(rinv[:], r_ps[:])
                bcast = spool.tile([128, cw], F16, tag="bcast")
                nc.gpsimd.partition_broadcast(bcast[:], rinv[:], channels=128)
                ot = opool.tile([HD, cw], F16, tag="o", name=f"o{ci}_{h}")
                nc.vector.tensor_copy(ot[:], o_ps[:])
                yt = ypool.tile([HD, cw], F16, tag="y", name=f"y{ci}_{h}")
                nc.vector.tensor_mul(yt[:], ot[:], bcast[:])
                y_sb.append(yt)
            pending_proj = (ci, y_sb)

        proj_chunk(*pending_proj)

        for ci, (start, cw) in enumerate(CHUNKS):
            nc.sync.dma_start(zc[:, start:start + cw], z_red[ci][:])
    nc.compile()
    return nc


def _tables():
    inv_freq = 1.0 / (ROPE_THETA ** (np.arange(0, HD, 2, dtype=np.float64) / HD))
    pos = np.arange(S, dtype=np.float64)
    f_half = np.outer(inv_freq, pos)                  # [64, S]
    freqs = np.concatenate([f_half, f_half], axis=0)  # [HD, S]
    # match reference numerics: cos/sin computed in float32 domain
    emb32 = freqs.astype(np.float32)
    cos_t = np.cos(emb32)
    sin_t = np.sin(emb32)
    scale = np.float32(HD ** -0.5)
    sgn = np.where(np.arange(HD) < HD // 2, -1.0, 1.0).astype(np.float32)[:, None]
    cosq = (cos_t * scale).astype(np.float16)
    sinq = (sin_t * sgn * scale).astype(np.float16)
    cosk = cos_t.astype(np.float16)
    sink = (sin_t * sgn).astype(np.float16)
    return cosq, sinq, cosk, sink


_NC_CACHE = {}


def _get_nc():
    if "nc" not in _NC_CACHE:
        _NC_CACHE["nc"] = _build()
    return _NC_CACHE["nc"]


def make_in_maps(x, W_attn, W_proj):
    x = np.asarray(x, dtype=np.float32)
    W_attn = np.asarray(W_attn, dtype=np.float32)
    W_proj = np.asarray(W_proj, dtype=np.float32)
    cosq, sinq, cosk, sink = _tables()
    tri = np.triu(np.ones((128, 128), np.float16))   # [jj, ii]: keep jj <= ii
    ones = np.ones((128, 1), np.float16)
    def pretile(w):  # [D, EL] -> [128, (d-tile, e)] partition-major tiling
        return np.ascontiguousarray(
            w.reshape(DT, 128, EL).transpose(1, 0, 2).reshape(128, DT * EL)
        ).astype(np.float16)

    in_maps = []
    for c in range(N_CORES):
        b, g = divmod(c, HPC)
        in_maps.append({
            "xT": np.ascontiguousarray(x[b].T).astype(np.float16),
            "wq": pretile(W_attn[:, g * EL:(g + 1) * EL]),
            "wk": pretile(W_attn[:, D + g * EL:D + (g + 1) * EL]),
            "wv": W_attn[:, 2 * D + g * EL:2 * D + (g + 1) * EL].astype(np.float16),
            "wp": W_proj[g * EL:(g + 1) * EL, :].astype(np.float16),
            "cosq": cosq, "sinq": sinq, "cosk": cosk, "sink": sink,
            "tri": tri, "ones": ones,
        })
    return in_maps


def assemble(results):
    out = np.empty((B, S, D), dtype=np.float32)
    for c in range(N_CORES):
        b, g = divmod(c, HPC)
        zT = np.asarray(results[c]["zc"], dtype=np.float32)
        out[b, :, g * EL:(g + 1) * EL] = zT.T
    return out


def kernel(x, W_attn, W_proj):
    nc = _get_nc()
    in_maps = make_in_maps(x, W_attn, W_proj)
    res = bass_utils.run_bass_kernel_spmd(
        nc, in_maps, core_ids=list(range(N_CORES)), trace=False)
    return assemble(res.results)


if __name__ == "__main__":
    rng = np.random.default_rng(0)
    x = rng.standard_normal((B, S, D)).astype(np.float32)
    W_attn = (rng.standard_normal((D, 3 * D)) * D ** -0.5).astype(np.float32)
    W_proj = (rng.standard_normal((D, D)) * D ** -0.5).astype(np.float32)
    out = kernel(x, W_attn, W_proj)
    print("out", out.shape, out.dtype, np.abs(out).mean())
